# revision 1
# baseline (speedup 1.0000x reference)
"""Trainium2 Bass kernel for the GRU session-decoder (nn_Decoder_12506944766179).

Strategy v2 (8 NeuronCores, SPMD, ZERO collectives):
  - Data-parallel over batch: core c owns batches 8c..8c+8.  Every core runs
    the full GRU recurrence for its 8 batches; there is no cross-core
    communication at all (the old per-step AllGather was ~270us/step).
  - The hidden state (2H = 2048 dims, host-permuted even/odd so max-pair
    pooling becomes h[0:1024] vs h[1024:2048]) is kept in a partition-packed
    layout [128, 512]: partition 32*j + b holds hidden quarter j of batch b.
    The recurrent matmul gh = h @ w_hh.T runs as 4 concurrent PE column
    tiles (tile_size 128x32, one per quarter), and all elementwise gate math
    runs at full 128-partition width on DVE/ACT.
  - w_hh is fp16, fully SBUF-resident ([128, 16*6144] = 192KiB/partition).
    PSUM accumulation is f32; the h state is kept f32, cast to fp16 only as
    the matmul stationary operand (hT).
  - Phase 0 precomputes gi = emb[x] @ w_ih.T (+biases) for all steps into
    DRAM (fp16, packed layout), plus xe + b2 for the output residual.
  - Phase 2 (after the recurrence) computes max-pair -> lin2 -> +xe ->
    out_embed over all (t, b) rows with full-width matmuls.
"""

import os
import sys

sys.path.insert(0, "/opt/trn_rl_repo")

import numpy as np

import concourse.bass as bass
import concourse.mybir as mybir
import concourse.tile as tile
from concourse import bacc
from concourse.masks import make_identity

V, E, SH, H, B, T = 10004, 512, 1024, 1024, 64, 128
H2 = 2 * H                # 2048 hidden
G = 3 * H2                # 6144 gate columns
NCORES = 8
BL = B // NCORES          # 8 batches per core
f32 = mybir.dt.float32
f16 = mybir.dt.float16
i32 = mybir.dt.int32
AF = mybir.ActivationFunctionType


def build(nsteps=T):
    nphases = int(os.environ.get("K_PHASES", "3"))
    nrows = nsteps * BL   # token rows per core, (t, b) order

    nc = bacc.Bacc("TRN2", target_bir_lowering=False, debug=False,
                   num_devices=NCORES)

    emb = nc.declare_dram_parameter("emb", [V, E], f32, isOutput=False)
    idx = nc.declare_dram_parameter("idx", [nrows, 1], i32, isOutput=False)
    sesT_d = nc.declare_dram_parameter("sesT", [SH, BL], f32, isOutput=False)
    w1T_d = nc.declare_dram_parameter("w1T", [SH, H2], f32, isOutput=False)
    b1p_d = nc.declare_dram_parameter("b1p", [128, 512], f32, isOutput=False)
    wihT_d = nc.declare_dram_parameter("wihT", [E, G], f16, isOutput=False)
    bgi_d = nc.declare_dram_parameter("bgi", [128, G], f32, isOutput=False)
    whhT_d = nc.declare_dram_parameter("whhT", [H2, G], f16, isOutput=False)
    bhhn_d = nc.declare_dram_parameter("bhhn", [128, 512], f16, isOutput=False)
    w2T_d = nc.declare_dram_parameter("w2T", [H, E], f16, isOutput=False)
    b2t_d = nc.declare_dram_parameter("b2t", [128, E], f32, isOutput=False)
    woutT_d = nc.declare_dram_parameter("woutT", [E, V], f16, isOutput=False)
    out = nc.declare_dram_parameter("out", [nrows, V], f32, isOutput=True)

    with tile.TileContext(nc) as tc:
        with (
            tc.tile_pool(name="wts", bufs=1) as wts,
            tc.tile_pool(name="dram", bufs=1, space="DRAM") as dram,
        ):
            # persistent small tiles
            ident = wts.tile([128, 128], f32, name="ident")
            make_identity(nc, ident[:])
            bhhn = wts.tile([128, 512], f16, name="bhhn")
            nc.sync.dma_start(bhhn[:], bhhn_d[:])
            # hT: stationary operand layout [128, (k:16, b:32)], fp16
            hT = wts.tile([128, 16 * 32], f16, name="hT")
            nc.gpsimd.memset(hT[:], 0.0)
            # h state ping-pong (f32, packed layout [128, 512])
            h_pp = [wts.tile([128, 512], f32, name=f"h{i}") for i in range(2)]

            gi_dram = dram.tile([nsteps * 128, 1536], f16, name="gi_dram")
            xe_dram = dram.tile([max(nrows, 128), E], f32, name="xe_dram")
            dec_dram = dram.tile([128, nsteps * 64], f16, name="dec_dram")

            def emit_transposes(h_tile, ps_pool, tag, bufs=2):
                """h_tile [128, 512] packed (f32) -> hT [128, (k,b<8)] fp16.

                Full-width (128,128)-mode transposes only: row/col-tiled
                transposes interleaved with col-tiled matmuls wedge the PE
                intermittently (quadrant tiling mode-switch bug).
                """
                ptr = ps_pool.tile([128, 512], f32, name="ptr", tag=tag,
                                   bufs=bufs)
                for cp in range(4):
                    nc.tensor.transpose(
                        ptr[:, 128 * cp:128 * (cp + 1)],
                        h_tile[:, 128 * cp:128 * (cp + 1)],
                        ident[:, :])
                for cp in range(4):
                    nc.vector.tensor_copy(
                        hT[:].rearrange("p (j c4 w) -> p j c4 w",
                                        j=4, c4=4)[:, :, cp, 0:BL],
                        ptr[:, 128 * cp:128 * (cp + 1)].rearrange(
                            "p (j bb) -> p j bb", j=4)[:, :, 0:BL])

            # ---------------- phase 0: h0, xe gather, gi precompute ---------
            with (
                tc.tile_pool(name="p0sb", bufs=1) as p0,
                tc.tile_pool(name="p0ps", bufs=1, space="PSUM") as p0ps,
            ):
                # h0 = tanh(ses @ w1p.T + b1p), packed layout
                ses_sb = p0.tile([128, 8 * BL], f32, name="ses_sb")
                nc.sync.dma_start(
                    ses_sb[:].rearrange("p (k b) -> p k b", k=8),
                    sesT_d.rearrange("(k p) b -> p k b", p=128))
                w1_sb = p0.tile([128, 8 * H2], f32, name="w1_sb")
                nc.sync.dma_start(
                    w1_sb[:].rearrange("p (k n) -> p k n", k=8),
                    w1T_d.rearrange("(k p) n -> p k n", p=128))
                b1p = p0.tile([128, 512], f32, name="b1p")
                nc.sync.dma_start(b1p[:], b1p_d[:])
                sesp = p0.tile([128, 8 * 32], f32, name="sesp")
                nc.gpsimd.memset(sesp[:], 0.0)
                h0lvl = int(os.environ.get("K_H0", "4"))
                if nphases >= 0:
                    if h0lvl >= 1:
                        nc.vector.tensor_copy(
                            sesp[:].rearrange("p (k w) -> p k w",
                                              k=8)[:, :, 0:BL],
                            ses_sb[:].rearrange("p (k b) -> p k b", k=8))
                    ps0 = p0ps.tile([128, 512], f32, name="ps0")
                    if h0lvl >= 2:
                        for k in range(8):
                            for j in range(4):
                                nc.tensor.matmul(
                                    ps0[32 * j:32 * (j + 1), :],
                                    sesp[:, 32 * k:32 * (k + 1)],
                                    w1_sb[:, k * H2 + 512 * j:
                                          k * H2 + 512 * (j + 1)],
                                    start=(k == 0), stop=(k == 7),
                                    tile_position=(0, 32 * j),
                                    skip_group_check=True)
                    if h0lvl >= 3:
                        nc.vector.tensor_add(ps0[:], ps0[:], b1p[:])
                        nc.scalar.activation(h_pp[0][:], ps0[:], AF.Tanh)
                    if h0lvl >= 4:
                        emit_transposes(h_pp[0], p0ps, "tr0", bufs=1)

                # gi for all rows
                wih = p0.tile([128, 4 * G], f16, name="wih")
                nc.sync.dma_start(
                    wih[:].rearrange("p (k n) -> p k n", k=4),
                    wihT_d.rearrange("(k p) n -> p k n", p=128))
                bgi = p0.tile([128, G], f32, name="bgi")
                nc.sync.dma_start(bgi[:], bgi_d[:])
                b2t = p0.tile([128, E], f32, name="b2t")
                nc.sync.dma_start(b2t[:], b2t_d[:])
                zrow = p0.tile([24, 4 * 1536], f16, name="zrow")
                nc.gpsimd.memset(zrow[:], 0.0)

                r0 = 0
                while nphases >= 1 and r0 < nrows:
                    R = min(128, nrows - r0)
                    ntl = R // BL
                    idxB = p0.tile([128, 1], i32, name="idxB", tag="idxB",
                                   bufs=2)
                    nc.sync.dma_start(idxB[0:R, :], idx[r0:r0 + R, :])
                    xeB = p0.tile([128, E], f32, name="xeB", tag="xeB", bufs=2)
                    nc.gpsimd.indirect_dma_start(
                        out=xeB[0:R, :], out_offset=None, in_=emb[:],
                        in_offset=bass.IndirectOffsetOnAxis(ap=idxB[0:R, :1],
                                                            axis=0))
                    # xe + b2 staged for the output residual
                    xeb2 = p0.tile([128, E], f32, name="xeb2", tag="xeb2",
                                   bufs=2)
                    nc.vector.tensor_add(xeb2[0:R, :], xeB[0:R, :],
                                         b2t[0:R, :])
                    nc.sync.dma_start(xe_dram[r0:r0 + R, :], xeb2[0:R, :])
                    # xe^T (fp16) for the gi matmul
                    ptx = p0ps.tile([128, 512], f32, name="ptx", tag="ptx",
                                    bufs=2)
                    for kc in range(4):
                        nc.tensor.transpose(
                            ptx[:, 128 * kc:128 * kc + R],
                            xeB[0:R, 128 * kc:128 * (kc + 1)],
                            ident[0:R, 0:R])
                    xeT = p0.tile([128, 512], f16, name="xeT", tag="xeT",
                                  bufs=2)
                    nc.vector.tensor_copy(
                        xeT[:].rearrange("p (kc r) -> p kc r", kc=4)[:, :, 0:R],
                        ptx[:].rearrange("p (kc r) -> p kc r", kc=4)[:, :, 0:R])
                    gi_sb = p0.tile([128, G], f16, name="gi_sb", tag="gi_sb",
                                    bufs=2)
                    for g in range(3):
                        psgi = p0ps.tile([128, 2048], f32, name="psgi",
                                         tag="psgi")
                        for kc in range(4):
                            for j in range(4):
                                nc.tensor.matmul(
                                    psgi[0:R, 512 * j:512 * (j + 1)],
                                    xeT[:, 128 * kc:128 * kc + R],
                                    wih[:, kc * G + g * H2 + 512 * j:
                                        kc * G + g * H2 + 512 * (j + 1)],
                                    start=(kc == 0), stop=(kc == 3))
                        nc.vector.tensor_add(
                            gi_sb[0:R, g * H2:(g + 1) * H2],
                            psgi[0:R, :], bgi[0:R, g * H2:(g + 1) * H2])
                    # store packed: step row 32j+b  <- gi row (tl*BL+b)
                    t0 = r0 // BL
                    for tl in range(ntl):
                        dst = gi_dram[(t0 + tl) * 128:(t0 + tl + 1) * 128, :]
                        nc.sync.dma_start(
                            dst.rearrange("(j bb) (g c) -> bb g j c",
                                          j=4, g=3)[0:BL],
                            gi_sb[BL * tl:BL * (tl + 1), :].rearrange(
                                "b (g j c) -> b g j c", g=3, j=4))
                        # zero the 24 pad rows of each 32-row block
                        nc.sync.dma_start(
                            dst.rearrange("(j bb) c -> bb j c", j=4)[BL:32],
                            zrow[:].rearrange("b (j c) -> b j c", j=4))
                    r0 += R

            # ---------------- phase 1: recurrence ---------------------------
            if nphases >= 2:
              with (
                tc.tile_pool(name="msb", bufs=1) as msb,
                tc.tile_pool(name="mps", bufs=1, space="PSUM") as mps,
              ):
                whh = msb.tile([128, 16 * G], f16, name="whh")
                nc.sync.dma_start(
                    whh[:].rearrange("p (k n) -> p k n", k=16),
                    whhT_d.rearrange("(k p) n -> p k n", p=128))

                h_cur = h_pp[0]
                for t in range(nsteps):
                    gi_t = msb.tile([128, 1536], f16, name="gi_t", tag="gi_t",
                                    bufs=1)
                    nc.sync.dma_start(gi_t[:],
                                      gi_dram[128 * t:128 * (t + 1), :])
                    ghrz = mps.tile([128, 1024], f32, name="ghrz", tag="ghrz",
                                    bufs=2)
                    ghn = mps.tile([128, 512], f32, name="ghn", tag="ghn",
                                   bufs=2)
                    for q in range(3):
                        dst = (ghrz if q < 2 else ghn)
                        c0 = 512 * q if q < 2 else 0
                        for k in range(16):
                            for j in range(4):
                                nc.tensor.matmul(
                                    dst[32 * j:32 * (j + 1), c0:c0 + 512],
                                    hT[:, 32 * k:32 * (k + 1)],
                                    whh[:, k * G + (j * 3 + q) * 512:
                                        k * G + (j * 3 + q) * 512 + 512],
                                    start=(k == 0), stop=(k == 15),
                                    tile_position=(0, 32 * j),
                                    skip_group_check=True)
                    # r, z
                    nc.vector.tensor_add(ghrz[:], ghrz[:], gi_t[:, 0:1024])
                    rz = msb.tile([128, 1024], f16, name="rz", tag="rz")
                    nc.scalar.activation(rz[:], ghrz[:], AF.Sigmoid)
                    # n
                    nc.vector.tensor_add(ghn[:], ghn[:], bhhn[:])
                    npre = msb.tile([128, 512], f16, name="npre", tag="npre")
                    nc.vector.tensor_mul(npre[:], rz[:, 0:512], ghn[:])
                    nc.vector.tensor_add(npre[:], npre[:], gi_t[:, 1024:1536])
                    ng = msb.tile([128, 512], f16, name="ng", tag="ng")
                    nc.scalar.activation(ng[:], npre[:], AF.Tanh)
                    dd = msb.tile([128, 512], f16, name="dd", tag="dd")
                    nc.vector.tensor_sub(dd[:], h_cur[:], ng[:])
                    nc.vector.tensor_mul(dd[:], rz[:, 512:1024], dd[:])
                    h_new = h_pp[(t + 1) % 2]
                    nc.vector.tensor_add(h_new[:], ng[:], dd[:])
                    emit_transposes(h_new, mps, "tr")
                    dct = msb.tile([128, 256], f16, name="dct", tag="dct",
                                   bufs=2)
                    nc.vector.tensor_max(dct[:], hT[:, 0:256], hT[:, 256:512])
                    nc.sync.dma_start(
                        dec_dram[:, 64 * t:64 * (t + 1)].rearrange(
                            "p (k b) -> p k b", k=8),
                        dct[:].rearrange("p (k w) -> p k w", k=8)[:, :, 0:BL])
                    h_cur = h_new

            # ---------------- phase 2: output projections --------------------
            if nphases >= 3:
              with (
                tc.tile_pool(name="p2sb", bufs=1) as p2,
                tc.tile_pool(name="p2ps", bufs=1, space="PSUM") as p2ps,
              ):
                wout = p2.tile([128, 4 * V], f16, name="wout")
                nc.sync.dma_start(
                    wout[:].rearrange("p (k n) -> p k n", k=4),
                    woutT_d.rearrange("(k p) n -> p k n", p=128))
                w2 = p2.tile([128, 8 * E], f16, name="w2")
                nc.sync.dma_start(
                    w2[:].rearrange("p (k n) -> p k n", k=8),
                    w2T_d.rearrange("(k p) n -> p k n", p=128))

                r0 = 0
                while r0 < nrows:
                    R = min(128, nrows - r0)
                    ntl = R // BL
                    t0 = r0 // BL
                    dec_sb = p2.tile([128, 16 * 64], f16, name="dec_sb",
                                     tag="dec_sb", bufs=2)
                    nc.sync.dma_start(dec_sb[:, 0:ntl * 64],
                                      dec_dram[:, t0 * 64:(t0 + ntl) * 64])
                    # repack (tl, k, b) -> (k, tl, b) so lhsT slices are
                    # contiguous
                    dec2_sb = p2.tile([128, 16 * 64], f16, name="dec2_sb",
                                      tag="dec2_sb", bufs=2)
                    nc.vector.tensor_copy(
                        dec2_sb[:, 0:ntl * 64].rearrange(
                            "p (k tl b) -> p k tl b", k=8, tl=ntl),
                        dec_sb[:, 0:ntl * 64].rearrange(
                            "p (tl k b) -> p k tl b", tl=ntl, k=8))
                    ps2 = p2ps.tile([128, 512], f32, name="ps2", tag="ps2",
                                    bufs=2)
                    for k in range(8):
                        nc.tensor.matmul(
                            ps2[0:R, :],
                            dec2_sb[:, k * ntl * BL:(k + 1) * ntl * BL],
                            w2[:, 512 * k:512 * (k + 1)],
                            start=(k == 0), stop=(k == 7))
                    xe_sb = p2.tile([128, E], f32, name="xe_sb", tag="xe_sb",
                                    bufs=2)
                    nc.sync.dma_start(xe_sb[0:R, :], xe_dram[r0:r0 + R, :])
                    dec2 = p2.tile([128, E], f32, name="dec2", tag="dec2",
                                   bufs=2)
                    nc.vector.tensor_add(dec2[0:R, :], ps2[0:R, :],
                                         xe_sb[0:R, :])
                    pst = p2ps.tile([128, 512], f32, name="pst", tag="pst",
                                    bufs=2)
                    for kc in range(4):
                        nc.tensor.transpose(
                            pst[:, 128 * kc:128 * kc + R],
                            dec2[0:R, 128 * kc:128 * (kc + 1)],
                            ident[0:R, 0:R])
                    d2T = p2.tile([128, 512], f16, name="d2T", tag="d2T",
                                  bufs=2)
                    nc.vector.tensor_copy(
                        d2T[:].rearrange("p (kc r) -> p kc r", kc=4)[:, :, 0:R],
                        pst[:].rearrange("p (kc r) -> p kc r", kc=4)[:, :, 0:R])
                    for vc in range(20):
                        n0 = 512 * vc
                        NN = min(512, V - n0)
                        psl = p2ps.tile([128, 512], f32, name="psl", tag="psl",
                                        bufs=2)
                        for kc in range(4):
                            nc.tensor.matmul(
                                psl[0:R, 0:NN],
                                d2T[:, 128 * kc:128 * kc + R],
                                wout[:, kc * V + n0:kc * V + n0 + NN],
                                start=(kc == 0), stop=(kc == 3))
                        lgs = p2.tile([128, 512], f32, name="lgs", tag="lgs",
                                      bufs=3)
                        nc.vector.tensor_copy(lgs[0:R, 0:NN], psl[0:R, 0:NN])
                        nc.sync.dma_start(out[r0:r0 + R, n0:n0 + NN],
                                          lgs[0:R, 0:NN])
                    r0 += R

    nc.compile()
    return nc


# ---------------------------------------------------------------------------
# host side
# ---------------------------------------------------------------------------

def _prep_inputs(ses_encoding, x, x_lens, emb_table, w1, b1, w_ih, w_hh,
                 b_ih, b_hh, w2, b2, w_out, nsteps=T):
    f = np.float32
    h = np.float16
    ses = np.asarray(ses_encoding, f)
    emb = np.ascontiguousarray(np.asarray(emb_table, f))
    w1 = np.asarray(w1, f)
    b1 = np.asarray(b1, f)
    w_ih = np.asarray(w_ih, f)
    w_hh = np.asarray(w_hh, f)
    b_ih = np.asarray(b_ih, f)
    b_hh = np.asarray(b_hh, f)
    w2 = np.asarray(w2, f)
    b2 = np.asarray(b2, f)
    w_out = np.asarray(w_out, f)
    x = np.asarray(x).astype(np.int32)

    hperm = np.concatenate([np.arange(0, H2, 2), np.arange(1, H2, 2)])

    # shared weights (identical on every core)
    w1T = np.ascontiguousarray(w1[hperm % H, :].T)                 # (SH, 2048)
    b1p = np.ascontiguousarray(
        np.repeat(b1[hperm % H].reshape(4, 512), 32, axis=0)).astype(f)
    gcols = np.concatenate([g * H2 + hperm for g in range(3)])     # (6144,)
    wihT = np.ascontiguousarray(w_ih[gcols, :].T.astype(h))        # (512, 6144)
    bias_v = (b_ih[gcols] +
              np.where(np.arange(G) < 2 * H2, b_hh[gcols], 0.0)).astype(f)
    bgi = np.ascontiguousarray(np.tile(bias_v, (128, 1)))          # (128, 6144)
    grows = np.concatenate([q * H2 + hperm[512 * j:512 * (j + 1)]
                            for j in range(4) for q in range(3)])  # (6144,)
    whhT = np.ascontiguousarray(w_hh[grows][:, hperm].T.astype(h)) # (2048,6144)
    bhhn = np.ascontiguousarray(
        np.repeat(b_hh[2 * H2 + hperm].reshape(4, 512), 32, axis=0)).astype(h)
    w2T = np.ascontiguousarray(w2.T.astype(h))                     # (1024, 512)
    b2t = np.ascontiguousarray(np.tile(b2.reshape(1, E), (128, 1))).astype(f)
    woutT = np.ascontiguousarray(w_out.T.astype(h))                # (512, 10004)

    in_maps = []
    for c in range(NCORES):
        xloc = x[BL * c:BL * (c + 1), :nsteps]                     # (8, t)
        idxs = np.ascontiguousarray(xloc.T.reshape(nsteps * BL, 1))
        sesT = np.ascontiguousarray(ses[0, BL * c:BL * (c + 1), :].T)
        in_maps.append(dict(
            emb=emb, idx=idxs, sesT=sesT, w1T=w1T, b1p=b1p,
            wihT=wihT, bgi=bgi, whhT=whhT, bhhn=bhhn, w2T=w2T, b2t=b2t,
            woutT=woutT))
    return in_maps


_CACHED = {}


def _get_runner(nsteps=T):
    key = nsteps
    if key not in _CACHED:
        nc = build(nsteps)
        _CACHED[key] = _SpmdRunner(nc, NCORES)
    return _CACHED[key]


class _SpmdRunner:
    def __init__(self, nc, n_cores):
        import jax
        import jax.numpy as jnp
        from jax.sharding import Mesh, PartitionSpec
        from jax.experimental.shard_map import shard_map
        from concourse.bass2jax import (_bass_exec_p, partition_id_tensor,
                                        install_neuronx_cc_hook)
        self.jax = jax
        self.jnp = jnp
        install_neuronx_cc_hook()
        self.nc = nc
        self.n_cores = n_cores
        in_names, out_names, out_avals = [], [], []
        pname = nc.partition_id_tensor.name if nc.partition_id_tensor else None
        for alloc in nc.m.functions[0].allocations:
            if not isinstance(alloc, mybir.MemoryLocationSet):
                continue
            name = alloc.memorylocations[0].name
            if alloc.kind == "ExternalInput":
                if name != pname:
                    in_names.append(name)
            elif alloc.kind == "ExternalOutput":
                out_names.append(name)
                out_avals.append(jax.core.ShapedArray(
                    tuple(alloc.tensor_shape), mybir.dt.np(alloc.dtype)))
        self.in_names, self.out_names, self.out_avals = \
            in_names, out_names, out_avals
        n_params, n_outs = len(in_names), len(out_avals)
        all_in = in_names + out_names + ([pname] if pname else [])

        def _body(*args):
            operands = list(args)
            if pname is not None:
                operands.append(partition_id_tensor())
            return tuple(_bass_exec_p.bind(
                *operands, out_avals=tuple(out_avals), in_names=tuple(all_in),
                out_names=tuple(out_names), lowering_input_output_aliases=(),
                sim_require_finite=False, sim_require_nnan=False, nc=nc))

        devices = jax.devices()[:n_cores]
        mesh = Mesh(np.asarray(devices), ("core",))
        self.donate = tuple(range(n_params, n_params + n_outs))
        self.sharded = jax.jit(
            shard_map(_body, mesh=mesh,
                      in_specs=(PartitionSpec("core"),) * (n_params + n_outs),
                      out_specs=(PartitionSpec("core"),) * n_outs,
                      check_rep=False),
            donate_argnums=self.donate, keep_unused=True)

    def set_inputs(self, in_maps):
        jax = self.jax
        per_core = [[np.ascontiguousarray(m[n]) for n in self.in_names]
                    for m in in_maps]
        concat = [np.concatenate([per_core[c][i] for c in range(self.n_cores)],
                                 axis=0) for i in range(len(self.in_names))]
        self._dev_in = [jax.device_put(a) for a in concat]
        for a in self._dev_in:
            a.block_until_ready()

    def _zeros(self):
        return [self.jnp.zeros((self.n_cores * av.shape[0], *av.shape[1:]),
                               av.dtype) for av in self.out_avals]

    def run_raw(self):
        outs = self.sharded(*self._dev_in, *self._zeros())
        for o in outs:
            o.block_until_ready()
        return outs

    def results(self):
        outs = self.run_raw()
        res = []
        for c in range(self.n_cores):
            res.append({n: np.asarray(outs[i]).reshape(
                self.n_cores, *self.out_avals[i].shape)[c]
                for i, n in enumerate(self.out_names)})
        return res

    def time(self, iters=10, warmup=2):
        import time as _t
        for _ in range(warmup):
            self.run_raw()
        ts = []
        for _ in range(iters):
            z = self._zeros()
            for zz in z:
                zz.block_until_ready()
            t0 = _t.perf_counter()
            outs = self.sharded(*self._dev_in, *z)
            for o in outs:
                o.block_until_ready()
            ts.append(_t.perf_counter() - t0)
        return min(ts), ts


def kernel(**inputs):
    nsteps = T
    runner = _get_runner(nsteps)
    in_maps = _prep_inputs(**inputs, nsteps=nsteps)
    runner.set_inputs(in_maps)
    res = runner.results()
    # per core: out [nsteps*BL, V] rows (t, b) -> full (b, t, v)
    parts = []
    for c in range(NCORES):
        o = res[c]["out"].reshape(nsteps, BL, V)
        parts.append(o.transpose(1, 0, 2))
    full = np.concatenate(parts, axis=0)                  # (B, T, V)
    return np.ascontiguousarray(full)



# revision 19
# speedup vs baseline: 5.8506x; 5.8506x over previous
"""Trainium2 Bass kernel for the GRU session-decoder (nn_Decoder_12506944766179).

Strategy v3 (8 NeuronCores, SPMD, zero collectives):
  - Data-parallel over batch: core c owns batches 8c..8c+8 and runs the full
    GRU recurrence for them locally; no cross-core communication.
  - Hidden state (2H = 2048, host-permuted even/odd so max-pair pooling is
    h[0:1024] vs h[1024:2048]) lives partition-packed [128, 512] fp16:
    partition 32*j + b holds hidden quarter j of batch b.  The recurrent
    matmul gh = h @ w_hh.T runs as 4 concurrent PE column tiles
    (tile_position col tiling, 128x32 each); gate math runs at full
    128-partition width on DVE/ACT in fp16.
  - w_hh is fp16 and fully SBUF-resident.  n-gate matmuls are emitted first
    so ghn post-processing overlaps the r/z matmuls.
  - gi = emb[x] @ w_ih.T (+ biases) is precomputed for all steps into DRAM
    in a compact [(t, j, b), 1536] fp16 layout: stores and loads move as
    4 contiguous 24KiB descriptors per step (the old padded layout burned
    ~100 1KiB fragments per step).
  - Phase 2 computes max-pair -> lin2 -> +xe -> out_embed with w_out
    streamed chunk-by-chunk from DRAM (vocab-chunk outer loop), so only a
    2048-col fp16 chunk is SBUF-resident at a time.
  - Host side: all device buffers are placed with NamedSharding once;
    outputs are donated, so a steady-state call does no resharding.
"""

import os
import sys

sys.path.insert(0, "/opt/trn_rl_repo")

import numpy as np

import concourse.bass as bass
import concourse.mybir as mybir
import concourse.tile as tile
from concourse import bacc
from concourse.masks import make_identity

V, E, SH, H, B, T = 10004, 512, 1024, 1024, 64, 128
H2 = 2 * H                # 2048 hidden
G = 3 * H2                # 6144 gate columns
NCORES = 8
BL = B // NCORES          # 8 batches per core
WOFF = 32 - BL            # real batch b sits at partition 32*j + WOFF + b
NVC = (V + 511) // 512    # 20 vocab chunks of 512 (last one padded)
f32 = mybir.dt.float32
f16 = mybir.dt.float16
i32 = mybir.dt.int32
AF = mybir.ActivationFunctionType


def build(nsteps=T):
    nphases = int(os.environ.get("K_PHASES", "3"))
    nrows = nsteps * BL   # token rows per core, (t, b) order

    nc = bacc.Bacc("TRN2", target_bir_lowering=False, debug=False,
                   num_devices=NCORES)

    emb = nc.declare_dram_parameter("emb", [V, E], f32, isOutput=False)
    idx = nc.declare_dram_parameter("idx", [nrows, 1], i32, isOutput=False)
    sesT_d = nc.declare_dram_parameter("sesT", [SH, BL], f32, isOutput=False)
    w1T_d = nc.declare_dram_parameter("w1T", [SH, H2], f32, isOutput=False)
    b1p_d = nc.declare_dram_parameter("b1p", [128, 512], f32, isOutput=False)
    wihT_d = nc.declare_dram_parameter("wihT", [E, G], f16, isOutput=False)
    bgi_d = nc.declare_dram_parameter("bgi", [128, G], f32, isOutput=False)
    whhT_d = nc.declare_dram_parameter("whhT", [H2, G], f16, isOutput=False)
    bhhn_d = nc.declare_dram_parameter("bhhn", [128, 512], f16, isOutput=False)
    w2T_d = nc.declare_dram_parameter("w2T", [H, E], f16, isOutput=False)
    b2t_d = nc.declare_dram_parameter("b2t", [128, E], f32, isOutput=False)
    wo2_d = nc.declare_dram_parameter("wo2", [NVC * 128, 2048], f16,
                                      isOutput=False)
    out = nc.declare_dram_parameter("out", [nrows, V], f32, isOutput=True)

    with tile.TileContext(nc) as tc:
        with (
            tc.tile_pool(name="wts", bufs=1) as wts,
            tc.tile_pool(name="dram", bufs=1, space="DRAM") as dram,
        ):
            # persistent small tiles
            ident16 = wts.tile([128, 128], f16, name="ident16")
            make_identity(nc, ident16[:])
            bhhn = wts.tile([128, 512], f16, name="bhhn")
            nc.sync.dma_start(bhhn[:], bhhn_d[:])
            # hT: stationary operand layout [128, (k:16, b:32)], fp16.
            # Pad columns (b >= BL) are zeroed once and never rewritten.
            hT = wts.tile([128, 16 * 32], f16, name="hT")
            nc.gpsimd.memset(hT[:], 0.0)
            # h state ping-pong (fp16, packed layout [128, 512])
            h_pp = [wts.tile([128, 512], f16, name=f"h{i}") for i in range(2)]

            gi_dram = dram.tile([nsteps * 32, 1536], f16, name="gi_dram")
            xe_dram = dram.tile([max(nrows, 128), E], f32, name="xe_dram")
            dec_dram = dram.tile([128, nsteps * 64], f16, name="dec_dram")

            def emit_transposes(h_tile, ps_pool, tag, bufs=2):
                """h_tile [128, 512] packed fp16 -> hT [128, (k, b<8)] fp16.

                4 full-width 128x128 PE transposes + one merged strided copy
                (chunk k = 4j + c4: hT col 32k+b <- ptr col 128*c4+32*j+b).
                """
                ptr = ps_pool.tile([128, 512], f16, name="ptr", tag=tag,
                                   bufs=bufs)
                for cp in range(4):
                    nc.tensor.transpose(
                        ptr[:, 128 * cp:128 * (cp + 1)],
                        h_tile[:, 128 * cp:128 * (cp + 1)],
                        ident16[:, :])
                nc.vector.tensor_copy(
                    hT[:].rearrange("p (j c4 w) -> p j c4 w",
                                    j=4, c4=4)[:, :, :, WOFF:32],
                    ptr[:].rearrange("p (c4 j w) -> p j c4 w",
                                     c4=4, j=4)[:, :, :, WOFF:32])

            # ---------------- phase 0: h0, xe gather, gi precompute ---------
            with (
                tc.tile_pool(name="p0sb", bufs=1) as p0,
                tc.tile_pool(name="p0ps", bufs=1, space="PSUM") as p0ps,
            ):
                ident = p0.tile([128, 128], f32, name="ident")
                make_identity(nc, ident[:])
                # h0 = tanh(ses @ w1p.T + b1p), packed layout
                ses_sb = p0.tile([128, 8 * BL], f32, name="ses_sb")
                nc.sync.dma_start(
                    ses_sb[:].rearrange("p (k b) -> p k b", k=8),
                    sesT_d.rearrange("(k p) b -> p k b", p=128))
                w1_sb = p0.tile([128, 8 * H2], f32, name="w1_sb")
                nc.sync.dma_start(
                    w1_sb[:].rearrange("p (k n) -> p k n", k=8),
                    w1T_d.rearrange("(k p) n -> p k n", p=128))
                b1p = p0.tile([128, 512], f32, name="b1p")
                nc.sync.dma_start(b1p[:], b1p_d[:])
                sesp = p0.tile([128, 8 * 32], f32, name="sesp")
                nc.gpsimd.memset(sesp[:], 0.0)
                nc.vector.tensor_copy(
                    sesp[:].rearrange("p (k w) -> p k w", k=8)[:, :, WOFF:32],
                    ses_sb[:].rearrange("p (k b) -> p k b", k=8))
                ps0 = p0ps.tile([128, 512], f32, name="ps0", bufs=1)
                for k in range(8):
                    for j in range(4):
                        nc.tensor.matmul(
                            ps0[32 * j:32 * (j + 1), :],
                            sesp[:, 32 * k:32 * (k + 1)],
                            w1_sb[:, k * H2 + 512 * j:
                                  k * H2 + 512 * (j + 1)],
                            start=(k == 0), stop=(k == 7),
                            tile_position=(0, 32 * j),
                            skip_group_check=True)
                nc.vector.tensor_add(ps0[:], ps0[:], b1p[:])
                nc.scalar.activation(h_pp[0][:], ps0[:], AF.Tanh)
                emit_transposes(h_pp[0], p0ps, "tr0", bufs=1)

                # gi for all rows
                wih = p0.tile([128, 4 * G], f16, name="wih")
                nc.sync.dma_start(
                    wih[:].rearrange("p (k n) -> p k n", k=4),
                    wihT_d.rearrange("(k p) n -> p k n", p=128))
                bgi = p0.tile([128, G], f32, name="bgi")
                nc.sync.dma_start(bgi[:], bgi_d[:])
                b2t = p0.tile([128, E], f32, name="b2t")
                nc.sync.dma_start(b2t[:], b2t_d[:])

                r0 = 0
                while nphases >= 1 and r0 < nrows:
                    R = min(128, nrows - r0)
                    ntl = R // BL
                    idxB = p0.tile([128, 1], i32, name="idxB", tag="idxB",
                                   bufs=2)
                    nc.sync.dma_start(idxB[0:R, :], idx[r0:r0 + R, :])
                    xeB = p0.tile([128, E], f32, name="xeB", tag="xeB", bufs=2)
                    nc.gpsimd.indirect_dma_start(
                        out=xeB[0:R, :], out_offset=None, in_=emb[:],
                        in_offset=bass.IndirectOffsetOnAxis(ap=idxB[0:R, :1],
                                                            axis=0))
                    # xe + b2 staged for the output residual
                    xeb2 = p0.tile([128, E], f32, name="xeb2", tag="xeb2",
                                   bufs=2)
                    nc.vector.tensor_add(xeb2[0:R, :], xeB[0:R, :],
                                         b2t[0:R, :])
                    nc.sync.dma_start(xe_dram[r0:r0 + R, :], xeb2[0:R, :])
                    # xe^T (fp16) for the gi matmul
                    ptx = p0ps.tile([128, 512], f32, name="ptx", tag="ptx",
                                    bufs=2)
                    for kc in range(4):
                        nc.tensor.transpose(
                            ptx[:, 128 * kc:128 * kc + R],
                            xeB[0:R, 128 * kc:128 * (kc + 1)],
                            ident[0:R, 0:R])
                    xeT = p0.tile([128, 512], f16, name="xeT", tag="xeT",
                                  bufs=2)
                    nc.vector.tensor_copy(
                        xeT[:].rearrange("p (kc r) -> p kc r", kc=4)[:, :, 0:R],
                        ptx[:].rearrange("p (kc r) -> p kc r", kc=4)[:, :, 0:R])
                    # gi_sb cols are (j, g, c) so per-(j, b) rows are
                    # 1536-contiguous for the compact store below
                    gi_sb = p0.tile([128, G], f16, name="gi_sb", tag="gi_sb",
                                    bufs=2)
                    for g in range(3):
                        psgi = p0ps.tile([128, 2048], f32, name="psgi",
                                         tag="psgi", bufs=1)
                        for kc in range(4):
                            for j in range(4):
                                nc.tensor.matmul(
                                    psgi[0:R, 512 * j:512 * (j + 1)],
                                    xeT[:, 128 * kc:128 * kc + R],
                                    wih[:, kc * G + g * H2 + 512 * j:
                                        kc * G + g * H2 + 512 * (j + 1)],
                                    start=(kc == 0), stop=(kc == 3))
                        nc.vector.tensor_add(
                            gi_sb[0:R, :].rearrange(
                                "r (j gg c) -> r j gg c", j=4, gg=3)[:, :, g],
                            psgi[0:R, :].rearrange("r (j c) -> r j c", j=4),
                            bgi[0:R, :].rearrange(
                                "r (j gg c) -> r j gg c", j=4, gg=3)[:, :, g])
                    # compact store: row t*32 + 4*b + j <- gi_sb row tl*BL+b,
                    # col slice j*1536 (8 contiguous 12KiB descriptors)
                    t0 = r0 // BL
                    for tl in range(ntl):
                        nc.sync.dma_start(
                            gi_dram[(t0 + tl) * 32:(t0 + tl + 1) * 32, :]
                            .rearrange("(b j) c -> b j c", b=BL),
                            gi_sb[BL * tl:BL * (tl + 1), :].rearrange(
                                "b (j c) -> b j c", j=4))
                    r0 += R

            # ---------------- phase 1: recurrence ---------------------------
            if nphases >= 2:
              with (
                tc.tile_pool(name="msb", bufs=1) as msb,
                tc.tile_pool(name="mps", bufs=1, space="PSUM") as mps,
              ):
                whh = msb.tile([128, 16 * G], f16, name="whh")
                nc.sync.dma_start(
                    whh[:].rearrange("p (k n) -> p k n", k=16),
                    whhT_d.rearrange("(k p) n -> p k n", p=128))

                h_cur = h_pp[0]
                for t in range(nsteps):
                    gi_t = msb.tile([128, 1536], f16, name="gi_t",
                                    tag="gi_t", bufs=2)
                    # zero the whole instance on the idle GPSIMD engine so
                    # pad partitions are initialized; the 4 contiguous-range
                    # loads below then overwrite the real partitions (real
                    # batch b lives at partition 32*j + WOFF + b)
                    nc.gpsimd.memset(gi_t[:], 0.0)
                    for j in range(4):
                        nc.sync.dma_start(
                            gi_t[32 * j + WOFF:32 * (j + 1), :],
                            gi_dram[32 * t:32 * (t + 1), :].rearrange(
                                "(b j) c -> b j c", b=BL)[:, j, :])
                    ghn = mps.tile([128, 512], f32, name="ghn", tag="ghn",
                                   bufs=2)
                    ghrz = mps.tile([128, 1024], f32, name="ghrz", tag="ghrz",
                                    bufs=2)
                    # n-gate first so its post-processing overlaps r/z MMs
                    for q in (2, 0, 1):
                        dst = ghn if q == 2 else ghrz
                        c0 = 0 if q == 2 else 512 * q
                        for k in range(16):
                            for j in range(4):
                                nc.tensor.matmul(
                                    dst[32 * j:32 * (j + 1), c0:c0 + 512],
                                    hT[:, 32 * k:32 * (k + 1)],
                                    whh[:, k * G + (j * 3 + q) * 512:
                                        k * G + (j * 3 + q) * 512 + 512],
                                    start=(k == 0), stop=(k == 15),
                                    tile_position=(0, 32 * j),
                                    skip_group_check=True)
                    # hn = ghn + b_hh_n  (runs during the r/z matmuls)
                    hn = msb.tile([128, 512], f16, name="hn", tag="hn")
                    nc.vector.tensor_add(hn[:], ghn[:], bhhn[:])
                    # r, z (merged in-place sigmoid over 1024)
                    rz = msb.tile([128, 1024], f16, name="rz", tag="rz")
                    nc.vector.tensor_add(rz[:], ghrz[:], gi_t[:, 0:1024])
                    nc.scalar.activation(rz[:], rz[:], AF.Sigmoid)
                    # n = tanh(r * hn + gi_n), in place
                    ng = msb.tile([128, 512], f16, name="ng", tag="ng")
                    nc.vector.tensor_mul(ng[:], rz[:, 0:512], hn[:])
                    nc.vector.tensor_add(ng[:], ng[:], gi_t[:, 1024:1536])
                    nc.scalar.activation(ng[:], ng[:], AF.Tanh)
                    # h_new = n + z * (h - n)  (dd reuses hn's buffer: hn's
                    # last read is the ng mul, which precedes dd's write)
                    dd = msb.tile([128, 512], f16, name="dd", tag="hn")
                    nc.vector.tensor_sub(dd[:], h_cur[:], ng[:])
                    nc.vector.tensor_mul(dd[:], rz[:, 512:1024], dd[:])
                    h_new = h_pp[(t + 1) % 2]
                    nc.vector.tensor_add(h_new[:], ng[:], dd[:])
                    emit_transposes(h_new, mps, "tr")
                    dct = msb.tile([128, 256], f16, name="dct", tag="dct",
                                   bufs=1)
                    nc.vector.tensor_max(dct[:], hT[:, 0:256], hT[:, 256:512])
                    nc.sync.dma_start(
                        dec_dram[:, 64 * t:64 * (t + 1)].rearrange(
                            "p (k b) -> p k b", k=8),
                        dct[:].rearrange("p (k w) -> p k w",
                                         k=8)[:, :, WOFF:32])
                    h_cur = h_new

            # ---------------- phase 2: output projections --------------------
            if nphases >= 3:
              with (
                tc.tile_pool(name="p2sb", bufs=1) as p2,
                tc.tile_pool(name="p2ps", bufs=1, space="PSUM") as p2ps,
              ):
                w2 = p2.tile([128, 8 * E], f16, name="w2")
                nc.sync.dma_start(
                    w2[:].rearrange("p (k n) -> p k n", k=8),
                    w2T_d.rearrange("(k p) n -> p k n", p=128))

                # stage A: per block, lin2 + xe residual, transposed fp16
                nblk = (nrows + 127) // 128
                d2T = [p2.tile([128, 512], f16, name=f"d2T{i}")
                       for i in range(nblk)]
                for blk in range(nblk):
                    r0 = blk * 128
                    R = min(128, nrows - r0)
                    ntl = R // BL
                    t0 = r0 // BL
                    dec_sb = p2.tile([128, 16 * 64], f16, name="dec_sb",
                                     tag="dec_sb", bufs=2)
                    nc.sync.dma_start(dec_sb[:, 0:ntl * 64],
                                      dec_dram[:, t0 * 64:(t0 + ntl) * 64])
                    # repack (tl, k, b) -> (k, tl, b) so lhsT slices are
                    # contiguous
                    dec2_sb = p2.tile([128, 16 * 64], f16, name="dec2_sb",
                                      tag="dec2_sb", bufs=2)
                    nc.vector.tensor_copy(
                        dec2_sb[:, 0:ntl * 64].rearrange(
                            "p (k tl b) -> p k tl b", k=8, tl=ntl),
                        dec_sb[:, 0:ntl * 64].rearrange(
                            "p (tl k b) -> p k tl b", tl=ntl, k=8))
                    ps2 = p2ps.tile([128, 512], f32, name="ps2", tag="ps2",
                                    bufs=2)
                    for k in range(8):
                        nc.tensor.matmul(
                            ps2[0:R, :],
                            dec2_sb[:, k * ntl * BL:(k + 1) * ntl * BL],
                            w2[:, 512 * k:512 * (k + 1)],
                            start=(k == 0), stop=(k == 7))
                    xe_sb = p2.tile([128, E], f32, name="xe_sb", tag="xe_sb",
                                    bufs=2)
                    nc.sync.dma_start(xe_sb[0:R, :], xe_dram[r0:r0 + R, :])
                    dec2 = p2.tile([128, E], f16, name="dec2", tag="dec2",
                                   bufs=2)
                    nc.vector.tensor_add(dec2[0:R, :], ps2[0:R, :],
                                         xe_sb[0:R, :])
                    pst = p2ps.tile([128, 512], f16, name="pst", tag="pst",
                                    bufs=2)
                    for kc in range(4):
                        nc.tensor.transpose(
                            pst[:, 128 * kc:128 * kc + R],
                            dec2[0:R, 128 * kc:128 * (kc + 1)],
                            ident16[0:R, 0:R])
                    nc.vector.tensor_copy(
                        d2T[blk][:].rearrange("p (kc r) -> p kc r",
                                              kc=4)[:, :, 0:R],
                        pst[:].rearrange("p (kc r) -> p kc r",
                                         kc=4)[:, :, 0:R])

                # stage B: vocab-chunk outer loop, w_out streamed from DRAM
                for vc in range(NVC):
                    n0 = 512 * vc
                    NN = min(512, V - n0)
                    wch = p2.tile([128, 2048], f16, name="wch", tag="wch",
                                  bufs=2)
                    nc.sync.dma_start(wch[:],
                                      wo2_d[128 * vc:128 * (vc + 1), :])
                    for blk in range(nblk):
                        r0 = blk * 128
                        R = min(128, nrows - r0)
                        psl = p2ps.tile([128, 512], f32, name="psl",
                                        tag="psl", bufs=2)
                        for kc in range(4):
                            nc.tensor.matmul(
                                psl[0:R, 0:512],
                                d2T[blk][:, 128 * kc:128 * kc + R],
                                wch[:, 512 * kc:512 * (kc + 1)],
                                start=(kc == 0), stop=(kc == 3))
                        lgs = p2.tile([128, 512], f32, name="lgs", tag="lgs",
                                      bufs=3)
                        nc.vector.tensor_copy(lgs[0:R, 0:NN], psl[0:R, 0:NN])
                        nc.sync.dma_start(out[r0:r0 + R, n0:n0 + NN],
                                          lgs[0:R, 0:NN])

    nc.compile()
    return nc


# ---------------------------------------------------------------------------
# host side
# ---------------------------------------------------------------------------

def _prep_inputs(ses_encoding, x, x_lens, emb_table, w1, b1, w_ih, w_hh,
                 b_ih, b_hh, w2, b2, w_out, nsteps=T):
    f = np.float32
    h = np.float16
    ses = np.asarray(ses_encoding, f)
    emb = np.ascontiguousarray(np.asarray(emb_table, f))
    w1 = np.asarray(w1, f)
    b1 = np.asarray(b1, f)
    w_ih = np.asarray(w_ih, f)
    w_hh = np.asarray(w_hh, f)
    b_ih = np.asarray(b_ih, f)
    b_hh = np.asarray(b_hh, f)
    w2 = np.asarray(w2, f)
    b2 = np.asarray(b2, f)
    w_out = np.asarray(w_out, f)
    x = np.asarray(x).astype(np.int32)

    hperm = np.concatenate([np.arange(0, H2, 2), np.arange(1, H2, 2)])

    # shared weights (identical on every core)
    w1T = np.ascontiguousarray(w1[hperm % H, :].T)                 # (SH, 2048)
    b1p = np.ascontiguousarray(
        np.repeat(b1[hperm % H].reshape(4, 512), 32, axis=0)).astype(f)
    gcols = np.concatenate([g * H2 + hperm for g in range(3)])     # (6144,)
    wihT = np.ascontiguousarray(w_ih[gcols, :].T.astype(h))        # (512, 6144)
    bias_v = (b_ih[gcols] +
              np.where(np.arange(G) < 2 * H2, b_hh[gcols], 0.0)).astype(f)
    # reorder (g, j, c) -> (j, g, c) for the compact gi layout
    bias_jgc = np.ascontiguousarray(
        bias_v.reshape(3, 4, 512).transpose(1, 0, 2).reshape(G))
    bgi = np.ascontiguousarray(np.tile(bias_jgc, (128, 1)))        # (128, 6144)
    grows = np.concatenate([q * H2 + hperm[512 * j:512 * (j + 1)]
                            for j in range(4) for q in range(3)])  # (6144,)
    whhT = np.ascontiguousarray(w_hh[grows][:, hperm].T.astype(h)) # (2048,6144)
    bhhn = np.ascontiguousarray(
        np.repeat(b_hh[2 * H2 + hperm].reshape(4, 512), 32, axis=0)).astype(h)
    w2T = np.ascontiguousarray(w2.T.astype(h))                     # (1024, 512)
    b2t = np.ascontiguousarray(np.tile(b2.reshape(1, E), (128, 1))).astype(f)
    # w_out in vocab-chunk layout: wo2[vc*128 + p, kc*512 + c]
    #   = w_out[512*vc + c, 128*kc + p]
    wo_pad = np.zeros((NVC * 512, E), f)
    wo_pad[:V] = w_out
    wo2 = np.ascontiguousarray(
        wo_pad.reshape(NVC, 512, 4, 128).transpose(0, 3, 2, 1)
        .reshape(NVC * 128, 2048).astype(h))

    in_maps = []
    for c in range(NCORES):
        xloc = x[BL * c:BL * (c + 1), :nsteps]                     # (8, t)
        idxs = np.ascontiguousarray(xloc.T.reshape(nsteps * BL, 1))
        sesT = np.ascontiguousarray(ses[0, BL * c:BL * (c + 1), :].T)
        in_maps.append(dict(
            emb=emb, idx=idxs, sesT=sesT, w1T=w1T, b1p=b1p,
            wihT=wihT, bgi=bgi, whhT=whhT, bhhn=bhhn, w2T=w2T, b2t=b2t,
            wo2=wo2))
    return in_maps


_CACHED = {}


def _get_runner(nsteps=T, nreps=1):
    key = (nsteps, nreps)
    if key not in _CACHED:
        nc = _CACHED.get(("nc", nsteps))
        if nc is None:
            nc = build(nsteps)
            _CACHED[("nc", nsteps)] = nc
        _CACHED[key] = _SpmdRunner(nc, NCORES, nreps=nreps)
    return _CACHED[key]


class _SpmdRunner:
    def __init__(self, nc, n_cores, nreps=1):
        import jax
        import jax.numpy as jnp
        from jax.sharding import Mesh, PartitionSpec, NamedSharding
        from jax.experimental.shard_map import shard_map
        from concourse.bass2jax import (_bass_exec_p, partition_id_tensor,
                                        install_neuronx_cc_hook)
        self.jax = jax
        self.jnp = jnp
        install_neuronx_cc_hook()
        self.nc = nc
        self.n_cores = n_cores
        self.nreps = nreps
        in_names, out_names, out_avals = [], [], []
        pname = nc.partition_id_tensor.name if nc.partition_id_tensor else None
        for alloc in nc.m.functions[0].allocations:
            if not isinstance(alloc, mybir.MemoryLocationSet):
                continue
            name = alloc.memorylocations[0].name
            if alloc.kind == "ExternalInput":
                if name != pname:
                    in_names.append(name)
            elif alloc.kind == "ExternalOutput":
                out_names.append(name)
                out_avals.append(jax.core.ShapedArray(
                    tuple(alloc.tensor_shape), mybir.dt.np(alloc.dtype)))
        self.in_names, self.out_names, self.out_avals = \
            in_names, out_names, out_avals
        n_params, n_outs = len(in_names), len(out_avals)
        all_in = in_names + out_names + ([pname] if pname else [])

        def _body(*args):
            # neuronx_cc_hook requires bass_exec operands to be the jit
            # parameters in exact order, so exactly one exec per jitted call.
            operands = list(args)
            if pname is not None:
                operands.append(partition_id_tensor())
            return tuple(_bass_exec_p.bind(
                *operands, out_avals=tuple(out_avals),
                in_names=tuple(all_in), out_names=tuple(out_names),
                lowering_input_output_aliases=(),
                sim_require_finite=False, sim_require_nnan=False, nc=nc))

        devices = jax.devices()[:n_cores]
        mesh = Mesh(np.asarray(devices), ("core",))
        # Shard inputs/outputs along axis 0 across the cores ONCE, outside
        # the dispatch path — otherwise every jitted call reshards ~500MB of
        # weights plus the output buffer from device 0 to all 8 cores.
        self.sharding = NamedSharding(mesh, PartitionSpec("core"))
        self.donate = tuple(range(n_params, n_params + n_outs))
        self.sharded = jax.jit(
            shard_map(_body, mesh=mesh,
                      in_specs=(PartitionSpec("core"),) * (n_params + n_outs),
                      out_specs=(PartitionSpec("core"),) * n_outs,
                      check_rep=False),
            donate_argnums=self.donate, keep_unused=True)

    def set_inputs(self, in_maps):
        jax = self.jax
        per_core = [[np.ascontiguousarray(m[n]) for n in self.in_names]
                    for m in in_maps]
        concat = [np.concatenate([per_core[c][i] for c in range(self.n_cores)],
                                 axis=0) for i in range(len(self.in_names))]
        self._dev_in = [jax.device_put(a, self.sharding) for a in concat]
        for a in self._dev_in:
            a.block_until_ready()

    def share_inputs(self, other):
        self._dev_in = other._dev_in

    def _zeros(self):
        return [self.jax.device_put(
            np.zeros((self.n_cores * av.shape[0], *av.shape[1:]), av.dtype),
            self.sharding) for av in self.out_avals]

    def run_raw(self):
        outs = self.sharded(*self._dev_in, *self._zeros())
        for o in outs:
            o.block_until_ready()
        return outs

    def results(self):
        outs = self.run_raw()
        res = []
        for c in range(self.n_cores):
            res.append({n: np.asarray(outs[i]).reshape(
                self.n_cores, *self.out_avals[i].shape)[c]
                for i, n in enumerate(self.out_names)})
        return res

    def time(self, iters=10, warmup=2):
        return self.time_chain(1, iters=iters, warmup=warmup)

    def time_chain(self, nrep, iters=10, warmup=2):
        """Wall time of `nrep` back-to-back executions per timed call.

        Calls are dispatched asynchronously, each feeding its outputs back
        as the next call's donated output buffers, and blocked once at the
        end — slope over nrep isolates on-device time if dispatch pipelines.
        """
        import time as _t
        for _ in range(warmup):
            self.run_raw()
        ts = []
        for _ in range(iters):
            outs = self._zeros()
            for zz in outs:
                zz.block_until_ready()
            t0 = _t.perf_counter()
            for _ in range(nrep):
                outs = self.sharded(*self._dev_in, *outs)
            for o in outs:
                o.block_until_ready()
            ts.append(_t.perf_counter() - t0)
        return min(ts), ts


def kernel(**inputs):
    nsteps = T
    runner = _get_runner(nsteps)
    in_maps = _prep_inputs(**inputs, nsteps=nsteps)
    runner.set_inputs(in_maps)
    res = runner.results()
    # per core: out [nsteps*BL, V] rows (t, b) -> full (b, t, v)
    parts = []
    for c in range(NCORES):
        o = res[c]["out"].reshape(nsteps, BL, V)
        parts.append(o.transpose(1, 0, 2))
    full = np.concatenate(parts, axis=0)                  # (B, T, V)
    return np.ascontiguousarray(full)


# revision 21
# speedup vs baseline: 7.6018x; 1.2993x over previous
"""Trainium2 Bass kernel for the GRU session-decoder (nn_Decoder_12506944766179).

Strategy v3 (8 NeuronCores, SPMD, zero collectives):
  - Data-parallel over batch: core c owns batches 8c..8c+8 and runs the full
    GRU recurrence for them locally; no cross-core communication.
  - Hidden state (2H = 2048, host-permuted even/odd so max-pair pooling is
    h[0:1024] vs h[1024:2048]) lives partition-packed [128, 512] fp16:
    partition 32*j + b holds hidden quarter j of batch b.  The recurrent
    matmul gh = h @ w_hh.T runs as 4 concurrent PE column tiles
    (tile_position col tiling, 128x32 each); gate math runs at full
    128-partition width on DVE/ACT in fp16.
  - w_hh is fp16 and fully SBUF-resident.  n-gate matmuls are emitted first
    so ghn post-processing overlaps the r/z matmuls.
  - gi = emb[x] @ w_ih.T (+ biases) is precomputed for all steps into DRAM
    in a compact [(t, j, b), 1536] fp16 layout: stores and loads move as
    4 contiguous 24KiB descriptors per step (the old padded layout burned
    ~100 1KiB fragments per step).
  - Phase 2 computes max-pair -> lin2 -> +xe -> out_embed with w_out
    streamed chunk-by-chunk from DRAM (vocab-chunk outer loop), so only a
    2048-col fp16 chunk is SBUF-resident at a time.
  - Host side: all device buffers are placed with NamedSharding once;
    outputs are donated, so a steady-state call does no resharding.
"""

import os
import sys

sys.path.insert(0, "/opt/trn_rl_repo")

import numpy as np

import concourse.bass as bass
import concourse.mybir as mybir
import concourse.tile as tile
from concourse import bacc
from concourse.masks import make_identity

V, E, SH, H, B, T = 10004, 512, 1024, 1024, 64, 128
H2 = 2 * H                # 2048 hidden
G = 3 * H2                # 6144 gate columns
NCORES = 8
BL = B // NCORES          # 8 batches per core
WOFF = 32 - BL            # real batch b sits at partition 32*j + WOFF + b
NVC = (V + 511) // 512    # 20 vocab chunks of 512 (last one padded)
f32 = mybir.dt.float32
f16 = mybir.dt.float16
i32 = mybir.dt.int32
AF = mybir.ActivationFunctionType


def build(nsteps=T):
    nphases = int(os.environ.get("K_PHASES", "3"))
    nrows = nsteps * BL   # token rows per core, (t, b) order

    nc = bacc.Bacc("TRN2", target_bir_lowering=False, debug=False,
                   num_devices=NCORES)

    emb = nc.declare_dram_parameter("emb", [V, E], f32, isOutput=False)
    idx = nc.declare_dram_parameter("idx", [nrows, 1], i32, isOutput=False)
    sesT_d = nc.declare_dram_parameter("sesT", [SH, BL], f32, isOutput=False)
    w1T_d = nc.declare_dram_parameter("w1T", [SH, H2], f32, isOutput=False)
    b1p_d = nc.declare_dram_parameter("b1p", [128, 512], f32, isOutput=False)
    wihT_d = nc.declare_dram_parameter("wihT", [E, G], f16, isOutput=False)
    bgi_d = nc.declare_dram_parameter("bgi", [128, G], f32, isOutput=False)
    whhT_d = nc.declare_dram_parameter("whhT", [H2, G], f16, isOutput=False)
    bhhn_d = nc.declare_dram_parameter("bhhn", [128, 512], f16, isOutput=False)
    w2T_d = nc.declare_dram_parameter("w2T", [H, E], f16, isOutput=False)
    b2t_d = nc.declare_dram_parameter("b2t", [128, E], f32, isOutput=False)
    wo2_d = nc.declare_dram_parameter("wo2", [NVC * 128, 2048], f16,
                                      isOutput=False)
    out = nc.declare_dram_parameter("out", [nrows, V], f32, isOutput=True)

    with tile.TileContext(nc) as tc:
        with (
            tc.tile_pool(name="wts", bufs=1) as wts,
            tc.tile_pool(name="dram", bufs=1, space="DRAM") as dram,
        ):
            # persistent small tiles
            ident16 = wts.tile([128, 128], f16, name="ident16")
            make_identity(nc, ident16[:])
            bhhn = wts.tile([128, 512], f16, name="bhhn")
            nc.sync.dma_start(bhhn[:], bhhn_d[:])
            # hT: stationary operand layout [128, (k:16, b:32)], fp16.
            # Pad columns (b >= BL) are zeroed once and never rewritten.
            hT = wts.tile([128, 16 * 32], f16, name="hT")
            nc.gpsimd.memset(hT[:], 0.0)
            # h state ping-pong (fp16, packed layout [128, 512])
            h_pp = [wts.tile([128, 512], f16, name=f"h{i}") for i in range(2)]

            gi_dram = dram.tile([nsteps * 32, 1536], f16, name="gi_dram")
            xe_dram = dram.tile([max(nrows, 128), E], f32, name="xe_dram")
            dec_dram = dram.tile([128, nsteps * 64], f16, name="dec_dram")

            def emit_transposes(h_tile, ps_pool, tag, bufs=2):
                """h_tile [128, 512] packed fp16 -> hT [128, (k, b<8)] fp16.

                4 full-width 128x128 PE transposes + one merged strided copy
                (chunk k = 4j + c4: hT col 32k+b <- ptr col 128*c4+32*j+b).
                """
                ptr = ps_pool.tile([128, 512], f16, name="ptr", tag=tag,
                                   bufs=bufs)
                for cp in range(4):
                    nc.tensor.transpose(
                        ptr[:, 128 * cp:128 * (cp + 1)],
                        h_tile[:, 128 * cp:128 * (cp + 1)],
                        ident16[:, :])
                nc.vector.tensor_copy(
                    hT[:].rearrange("p (j c4 w) -> p j c4 w",
                                    j=4, c4=4)[:, :, :, WOFF:32],
                    ptr[:].rearrange("p (c4 j w) -> p j c4 w",
                                     c4=4, j=4)[:, :, :, WOFF:32])

            # ---------------- phase 0: h0, xe gather, gi precompute ---------
            with (
                tc.tile_pool(name="p0sb", bufs=1) as p0,
                tc.tile_pool(name="p0ps", bufs=1, space="PSUM") as p0ps,
            ):
                ident = p0.tile([128, 128], f32, name="ident")
                make_identity(nc, ident[:])
                # h0 = tanh(ses @ w1p.T + b1p), packed layout
                ses_sb = p0.tile([128, 8 * BL], f32, name="ses_sb")
                nc.sync.dma_start(
                    ses_sb[:].rearrange("p (k b) -> p k b", k=8),
                    sesT_d.rearrange("(k p) b -> p k b", p=128))
                w1_sb = p0.tile([128, 8 * H2], f32, name="w1_sb")
                nc.sync.dma_start(
                    w1_sb[:].rearrange("p (k n) -> p k n", k=8),
                    w1T_d.rearrange("(k p) n -> p k n", p=128))
                b1p = p0.tile([128, 512], f32, name="b1p")
                nc.sync.dma_start(b1p[:], b1p_d[:])
                sesp = p0.tile([128, 8 * 32], f32, name="sesp")
                nc.gpsimd.memset(sesp[:], 0.0)
                nc.vector.tensor_copy(
                    sesp[:].rearrange("p (k w) -> p k w", k=8)[:, :, WOFF:32],
                    ses_sb[:].rearrange("p (k b) -> p k b", k=8))
                ps0 = p0ps.tile([128, 512], f32, name="ps0", bufs=1)
                for k in range(8):
                    for j in range(4):
                        nc.tensor.matmul(
                            ps0[32 * j:32 * (j + 1), :],
                            sesp[:, 32 * k:32 * (k + 1)],
                            w1_sb[:, k * H2 + 512 * j:
                                  k * H2 + 512 * (j + 1)],
                            start=(k == 0), stop=(k == 7),
                            tile_position=(0, 32 * j),
                            skip_group_check=True)
                nc.vector.tensor_add(ps0[:], ps0[:], b1p[:])
                nc.scalar.activation(h_pp[0][:], ps0[:], AF.Tanh)
                emit_transposes(h_pp[0], p0ps, "tr0", bufs=1)

                # gi for all rows
                wih = p0.tile([128, 4 * G], f16, name="wih")
                nc.sync.dma_start(
                    wih[:].rearrange("p (k n) -> p k n", k=4),
                    wihT_d.rearrange("(k p) n -> p k n", p=128))
                bgi = p0.tile([128, G], f32, name="bgi")
                nc.sync.dma_start(bgi[:], bgi_d[:])
                b2t = p0.tile([128, E], f32, name="b2t")
                nc.sync.dma_start(b2t[:], b2t_d[:])

                r0 = 0
                while nphases >= 1 and r0 < nrows:
                    R = min(128, nrows - r0)
                    ntl = R // BL
                    idxB = p0.tile([128, 1], i32, name="idxB", tag="idxB",
                                   bufs=2)
                    nc.sync.dma_start(idxB[0:R, :], idx[r0:r0 + R, :])
                    xeB = p0.tile([128, E], f32, name="xeB", tag="xeB", bufs=2)
                    nc.gpsimd.indirect_dma_start(
                        out=xeB[0:R, :], out_offset=None, in_=emb[:],
                        in_offset=bass.IndirectOffsetOnAxis(ap=idxB[0:R, :1],
                                                            axis=0))
                    # xe + b2 staged for the output residual
                    xeb2 = p0.tile([128, E], f32, name="xeb2", tag="xeb2",
                                   bufs=2)
                    nc.vector.tensor_add(xeb2[0:R, :], xeB[0:R, :],
                                         b2t[0:R, :])
                    nc.sync.dma_start(xe_dram[r0:r0 + R, :], xeb2[0:R, :])
                    # xe^T (fp16) for the gi matmul
                    ptx = p0ps.tile([128, 512], f32, name="ptx", tag="ptx",
                                    bufs=2)
                    for kc in range(4):
                        nc.tensor.transpose(
                            ptx[:, 128 * kc:128 * kc + R],
                            xeB[0:R, 128 * kc:128 * (kc + 1)],
                            ident[0:R, 0:R])
                    xeT = p0.tile([128, 512], f16, name="xeT", tag="xeT",
                                  bufs=2)
                    nc.vector.tensor_copy(
                        xeT[:].rearrange("p (kc r) -> p kc r", kc=4)[:, :, 0:R],
                        ptx[:].rearrange("p (kc r) -> p kc r", kc=4)[:, :, 0:R])
                    # gi_sb cols are (j, g, c) so per-(j, b) rows are
                    # 1536-contiguous for the compact store below
                    gi_sb = p0.tile([128, G], f16, name="gi_sb", tag="gi_sb",
                                    bufs=2)
                    for g in range(3):
                        psgi = p0ps.tile([128, 2048], f32, name="psgi",
                                         tag="psgi", bufs=1)
                        for kc in range(4):
                            for j in range(4):
                                nc.tensor.matmul(
                                    psgi[0:R, 512 * j:512 * (j + 1)],
                                    xeT[:, 128 * kc:128 * kc + R],
                                    wih[:, kc * G + g * H2 + 512 * j:
                                        kc * G + g * H2 + 512 * (j + 1)],
                                    start=(kc == 0), stop=(kc == 3))
                        nc.vector.tensor_add(
                            gi_sb[0:R, :].rearrange(
                                "r (j gg c) -> r j gg c", j=4, gg=3)[:, :, g],
                            psgi[0:R, :].rearrange("r (j c) -> r j c", j=4),
                            bgi[0:R, :].rearrange(
                                "r (j gg c) -> r j gg c", j=4, gg=3)[:, :, g])
                    # compact store: row t*32 + 4*b + j <- gi_sb row tl*BL+b,
                    # col slice j*1536 (8 contiguous 12KiB descriptors)
                    t0 = r0 // BL
                    for tl in range(ntl):
                        nc.sync.dma_start(
                            gi_dram[(t0 + tl) * 32:(t0 + tl + 1) * 32, :]
                            .rearrange("(b j) c -> b j c", b=BL),
                            gi_sb[BL * tl:BL * (tl + 1), :].rearrange(
                                "b (j c) -> b j c", j=4))
                    r0 += R

            # ---------------- phase 1: recurrence ---------------------------
            if nphases >= 2:
              with (
                tc.tile_pool(name="msb", bufs=1) as msb,
                tc.tile_pool(name="mps", bufs=1, space="PSUM") as mps,
              ):
                whh = msb.tile([128, 16 * G], f16, name="whh")
                nc.sync.dma_start(
                    whh[:].rearrange("p (k n) -> p k n", k=16),
                    whhT_d.rearrange("(k p) n -> p k n", p=128))

                h_cur = h_pp[0]
                for t in range(nsteps):
                    gi_t = msb.tile([128, 1536], f16, name="gi_t",
                                    tag="gi_t", bufs=2)
                    # zero the whole instance on the idle GPSIMD engine so
                    # pad partitions are initialized; the 4 contiguous-range
                    # loads below then overwrite the real partitions (real
                    # batch b lives at partition 32*j + WOFF + b)
                    nc.gpsimd.memset(gi_t[:], 0.0)
                    for j in range(4):
                        nc.sync.dma_start(
                            gi_t[32 * j + WOFF:32 * (j + 1), :],
                            gi_dram[32 * t:32 * (t + 1), :].rearrange(
                                "(b j) c -> b j c", b=BL)[:, j, :])
                    ghn = mps.tile([128, 512], f32, name="ghn", tag="ghn",
                                   bufs=2)
                    ghrz = mps.tile([128, 1024], f32, name="ghrz", tag="ghrz",
                                    bufs=2)

                    def gate_mms(dst, c0, q, rhs0):
                        # open the accumulation group with a full-width
                        # identity matmul that injects the bias/gi term into
                        # PSUM (removes the DVE add from the critical path),
                        # then accumulate the 16 col-tiled h @ w_hh chunks
                        nc.tensor.matmul(dst[:, c0:c0 + 512], ident16[:, :],
                                         rhs0, start=True, stop=False,
                                         skip_group_check=True)
                        for k in range(16):
                            for j in range(4):
                                nc.tensor.matmul(
                                    dst[32 * j:32 * (j + 1), c0:c0 + 512],
                                    hT[:, 32 * k:32 * (k + 1)],
                                    whh[:, k * G + (j * 3 + q) * 512:
                                        k * G + (j * 3 + q) * 512 + 512],
                                    start=False, stop=(k == 15),
                                    tile_position=(0, 32 * j),
                                    skip_group_check=True)

                    # group order (n, r, z): the n/r-dependent chain
                    # (sigmoid -> n pre-act -> tanh) overlaps the z matmuls,
                    # so only z-sigmoid + 2 DVE ops trail the last matmul
                    gate_mms(ghn, 0, 2, bhhn[:])
                    gate_mms(ghrz, 0, 0, gi_t[:, 0:512])
                    r = msb.tile([128, 512], f16, name="r", tag="r")
                    nc.scalar.activation(r[:], ghrz[:, 0:512], AF.Sigmoid)
                    # n = tanh(r * (ghn + b_hh_n) + gi_n), in place
                    ng = msb.tile([128, 512], f16, name="ng", tag="ng")
                    nc.vector.tensor_mul(ng[:], r[:], ghn[:])
                    nc.vector.tensor_add(ng[:], ng[:], gi_t[:, 1024:1536])
                    nc.scalar.activation(ng[:], ng[:], AF.Tanh)
                    # dd = h - n can also run during the z matmuls
                    dd = msb.tile([128, 512], f16, name="dd", tag="dd")
                    nc.vector.tensor_sub(dd[:], h_cur[:], ng[:])
                    gate_mms(ghrz, 512, 1, gi_t[:, 512:1024])
                    z = msb.tile([128, 512], f16, name="z", tag="z")
                    nc.scalar.activation(z[:], ghrz[:, 512:1024], AF.Sigmoid)
                    # h_new = n + z * (h - n)
                    nc.vector.tensor_mul(dd[:], z[:], dd[:])
                    h_new = h_pp[(t + 1) % 2]
                    nc.vector.tensor_add(h_new[:], ng[:], dd[:])
                    emit_transposes(h_new, mps, "tr")
                    dct = msb.tile([128, 256], f16, name="dct", tag="dct",
                                   bufs=1)
                    nc.vector.tensor_max(dct[:], hT[:, 0:256], hT[:, 256:512])
                    nc.sync.dma_start(
                        dec_dram[:, 64 * t:64 * (t + 1)].rearrange(
                            "p (k b) -> p k b", k=8),
                        dct[:].rearrange("p (k w) -> p k w",
                                         k=8)[:, :, WOFF:32])
                    h_cur = h_new

            # ---------------- phase 2: output projections --------------------
            if nphases >= 3:
              with (
                tc.tile_pool(name="p2sb", bufs=1) as p2,
                tc.tile_pool(name="p2ps", bufs=1, space="PSUM") as p2ps,
              ):
                w2 = p2.tile([128, 8 * E], f16, name="w2")
                nc.sync.dma_start(
                    w2[:].rearrange("p (k n) -> p k n", k=8),
                    w2T_d.rearrange("(k p) n -> p k n", p=128))

                # stage A: per block, lin2 + xe residual, transposed fp16
                nblk = (nrows + 127) // 128
                d2T = [p2.tile([128, 512], f16, name=f"d2T{i}")
                       for i in range(nblk)]
                for blk in range(nblk):
                    r0 = blk * 128
                    R = min(128, nrows - r0)
                    ntl = R // BL
                    t0 = r0 // BL
                    dec_sb = p2.tile([128, 16 * 64], f16, name="dec_sb",
                                     tag="dec_sb", bufs=2)
                    nc.sync.dma_start(dec_sb[:, 0:ntl * 64],
                                      dec_dram[:, t0 * 64:(t0 + ntl) * 64])
                    # repack (tl, k, b) -> (k, tl, b) so lhsT slices are
                    # contiguous
                    dec2_sb = p2.tile([128, 16 * 64], f16, name="dec2_sb",
                                      tag="dec2_sb", bufs=2)
                    nc.vector.tensor_copy(
                        dec2_sb[:, 0:ntl * 64].rearrange(
                            "p (k tl b) -> p k tl b", k=8, tl=ntl),
                        dec_sb[:, 0:ntl * 64].rearrange(
                            "p (tl k b) -> p k tl b", tl=ntl, k=8))
                    ps2 = p2ps.tile([128, 512], f32, name="ps2", tag="ps2",
                                    bufs=2)
                    for k in range(8):
                        nc.tensor.matmul(
                            ps2[0:R, :],
                            dec2_sb[:, k * ntl * BL:(k + 1) * ntl * BL],
                            w2[:, 512 * k:512 * (k + 1)],
                            start=(k == 0), stop=(k == 7))
                    xe_sb = p2.tile([128, E], f32, name="xe_sb", tag="xe_sb",
                                    bufs=2)
                    nc.sync.dma_start(xe_sb[0:R, :], xe_dram[r0:r0 + R, :])
                    dec2 = p2.tile([128, E], f16, name="dec2", tag="dec2",
                                   bufs=2)
                    nc.vector.tensor_add(dec2[0:R, :], ps2[0:R, :],
                                         xe_sb[0:R, :])
                    pst = p2ps.tile([128, 512], f16, name="pst", tag="pst",
                                    bufs=2)
                    for kc in range(4):
                        nc.tensor.transpose(
                            pst[:, 128 * kc:128 * kc + R],
                            dec2[0:R, 128 * kc:128 * (kc + 1)],
                            ident16[0:R, 0:R])
                    nc.vector.tensor_copy(
                        d2T[blk][:].rearrange("p (kc r) -> p kc r",
                                              kc=4)[:, :, 0:R],
                        pst[:].rearrange("p (kc r) -> p kc r",
                                         kc=4)[:, :, 0:R])

                # stage B: vocab-chunk outer loop, w_out streamed from DRAM
                for vc in range(NVC):
                    n0 = 512 * vc
                    NN = min(512, V - n0)
                    wch = p2.tile([128, 2048], f16, name="wch", tag="wch",
                                  bufs=2)
                    nc.sync.dma_start(wch[:],
                                      wo2_d[128 * vc:128 * (vc + 1), :])
                    for blk in range(nblk):
                        r0 = blk * 128
                        R = min(128, nrows - r0)
                        psl = p2ps.tile([128, 512], f32, name="psl",
                                        tag="psl", bufs=2)
                        for kc in range(4):
                            nc.tensor.matmul(
                                psl[0:R, 0:512],
                                d2T[blk][:, 128 * kc:128 * kc + R],
                                wch[:, 512 * kc:512 * (kc + 1)],
                                start=(kc == 0), stop=(kc == 3))
                        lgs = p2.tile([128, 512], f32, name="lgs", tag="lgs",
                                      bufs=3)
                        nc.vector.tensor_copy(lgs[0:R, 0:NN], psl[0:R, 0:NN])
                        nc.sync.dma_start(out[r0:r0 + R, n0:n0 + NN],
                                          lgs[0:R, 0:NN])

    nc.compile()
    return nc


# ---------------------------------------------------------------------------
# host side
# ---------------------------------------------------------------------------

def _prep_inputs(ses_encoding, x, x_lens, emb_table, w1, b1, w_ih, w_hh,
                 b_ih, b_hh, w2, b2, w_out, nsteps=T):
    f = np.float32
    h = np.float16
    ses = np.asarray(ses_encoding, f)
    emb = np.ascontiguousarray(np.asarray(emb_table, f))
    w1 = np.asarray(w1, f)
    b1 = np.asarray(b1, f)
    w_ih = np.asarray(w_ih, f)
    w_hh = np.asarray(w_hh, f)
    b_ih = np.asarray(b_ih, f)
    b_hh = np.asarray(b_hh, f)
    w2 = np.asarray(w2, f)
    b2 = np.asarray(b2, f)
    w_out = np.asarray(w_out, f)
    x = np.asarray(x).astype(np.int32)

    hperm = np.concatenate([np.arange(0, H2, 2), np.arange(1, H2, 2)])

    # shared weights (identical on every core)
    w1T = np.ascontiguousarray(w1[hperm % H, :].T)                 # (SH, 2048)
    b1p = np.ascontiguousarray(
        np.repeat(b1[hperm % H].reshape(4, 512), 32, axis=0)).astype(f)
    gcols = np.concatenate([g * H2 + hperm for g in range(3)])     # (6144,)
    wihT = np.ascontiguousarray(w_ih[gcols, :].T.astype(h))        # (512, 6144)
    bias_v = (b_ih[gcols] +
              np.where(np.arange(G) < 2 * H2, b_hh[gcols], 0.0)).astype(f)
    # reorder (g, j, c) -> (j, g, c) for the compact gi layout
    bias_jgc = np.ascontiguousarray(
        bias_v.reshape(3, 4, 512).transpose(1, 0, 2).reshape(G))
    bgi = np.ascontiguousarray(np.tile(bias_jgc, (128, 1)))        # (128, 6144)
    grows = np.concatenate([q * H2 + hperm[512 * j:512 * (j + 1)]
                            for j in range(4) for q in range(3)])  # (6144,)
    whhT = np.ascontiguousarray(w_hh[grows][:, hperm].T.astype(h)) # (2048,6144)
    bhhn = np.ascontiguousarray(
        np.repeat(b_hh[2 * H2 + hperm].reshape(4, 512), 32, axis=0)).astype(h)
    w2T = np.ascontiguousarray(w2.T.astype(h))                     # (1024, 512)
    b2t = np.ascontiguousarray(np.tile(b2.reshape(1, E), (128, 1))).astype(f)
    # w_out in vocab-chunk layout: wo2[vc*128 + p, kc*512 + c]
    #   = w_out[512*vc + c, 128*kc + p]
    wo_pad = np.zeros((NVC * 512, E), f)
    wo_pad[:V] = w_out
    wo2 = np.ascontiguousarray(
        wo_pad.reshape(NVC, 512, 4, 128).transpose(0, 3, 2, 1)
        .reshape(NVC * 128, 2048).astype(h))

    in_maps = []
    for c in range(NCORES):
        xloc = x[BL * c:BL * (c + 1), :nsteps]                     # (8, t)
        idxs = np.ascontiguousarray(xloc.T.reshape(nsteps * BL, 1))
        sesT = np.ascontiguousarray(ses[0, BL * c:BL * (c + 1), :].T)
        in_maps.append(dict(
            emb=emb, idx=idxs, sesT=sesT, w1T=w1T, b1p=b1p,
            wihT=wihT, bgi=bgi, whhT=whhT, bhhn=bhhn, w2T=w2T, b2t=b2t,
            wo2=wo2))
    return in_maps


_CACHED = {}


def _get_runner(nsteps=T, nreps=1):
    key = (nsteps, nreps)
    if key not in _CACHED:
        nc = _CACHED.get(("nc", nsteps))
        if nc is None:
            nc = build(nsteps)
            _CACHED[("nc", nsteps)] = nc
        _CACHED[key] = _SpmdRunner(nc, NCORES, nreps=nreps)
    return _CACHED[key]


class _SpmdRunner:
    def __init__(self, nc, n_cores, nreps=1):
        import jax
        import jax.numpy as jnp
        from jax.sharding import Mesh, PartitionSpec, NamedSharding
        from jax.experimental.shard_map import shard_map
        from concourse.bass2jax import (_bass_exec_p, partition_id_tensor,
                                        install_neuronx_cc_hook)
        self.jax = jax
        self.jnp = jnp
        install_neuronx_cc_hook()
        self.nc = nc
        self.n_cores = n_cores
        self.nreps = nreps
        in_names, out_names, out_avals = [], [], []
        pname = nc.partition_id_tensor.name if nc.partition_id_tensor else None
        for alloc in nc.m.functions[0].allocations:
            if not isinstance(alloc, mybir.MemoryLocationSet):
                continue
            name = alloc.memorylocations[0].name
            if alloc.kind == "ExternalInput":
                if name != pname:
                    in_names.append(name)
            elif alloc.kind == "ExternalOutput":
                out_names.append(name)
                out_avals.append(jax.core.ShapedArray(
                    tuple(alloc.tensor_shape), mybir.dt.np(alloc.dtype)))
        self.in_names, self.out_names, self.out_avals = \
            in_names, out_names, out_avals
        n_params, n_outs = len(in_names), len(out_avals)
        all_in = in_names + out_names + ([pname] if pname else [])

        def _body(*args):
            # neuronx_cc_hook requires bass_exec operands to be the jit
            # parameters in exact order, so exactly one exec per jitted call.
            operands = list(args)
            if pname is not None:
                operands.append(partition_id_tensor())
            return tuple(_bass_exec_p.bind(
                *operands, out_avals=tuple(out_avals),
                in_names=tuple(all_in), out_names=tuple(out_names),
                lowering_input_output_aliases=(),
                sim_require_finite=False, sim_require_nnan=False, nc=nc))

        devices = jax.devices()[:n_cores]
        mesh = Mesh(np.asarray(devices), ("core",))
        # Shard inputs/outputs along axis 0 across the cores ONCE, outside
        # the dispatch path — otherwise every jitted call reshards ~500MB of
        # weights plus the output buffer from device 0 to all 8 cores.
        self.sharding = NamedSharding(mesh, PartitionSpec("core"))
        self.donate = tuple(range(n_params, n_params + n_outs))
        self.sharded = jax.jit(
            shard_map(_body, mesh=mesh,
                      in_specs=(PartitionSpec("core"),) * (n_params + n_outs),
                      out_specs=(PartitionSpec("core"),) * n_outs,
                      check_rep=False),
            donate_argnums=self.donate, keep_unused=True)

    def set_inputs(self, in_maps):
        jax = self.jax
        per_core = [[np.ascontiguousarray(m[n]) for n in self.in_names]
                    for m in in_maps]
        concat = [np.concatenate([per_core[c][i] for c in range(self.n_cores)],
                                 axis=0) for i in range(len(self.in_names))]
        self._dev_in = [jax.device_put(a, self.sharding) for a in concat]
        for a in self._dev_in:
            a.block_until_ready()

    def share_inputs(self, other):
        self._dev_in = other._dev_in

    def _zeros(self):
        return [self.jax.device_put(
            np.zeros((self.n_cores * av.shape[0], *av.shape[1:]), av.dtype),
            self.sharding) for av in self.out_avals]

    def run_raw(self):
        outs = self.sharded(*self._dev_in, *self._zeros())
        for o in outs:
            o.block_until_ready()
        return outs

    def results(self):
        outs = self.run_raw()
        res = []
        for c in range(self.n_cores):
            res.append({n: np.asarray(outs[i]).reshape(
                self.n_cores, *self.out_avals[i].shape)[c]
                for i, n in enumerate(self.out_names)})
        return res

    def time(self, iters=10, warmup=2):
        return self.time_chain(1, iters=iters, warmup=warmup)

    def time_chain(self, nrep, iters=10, warmup=2):
        """Wall time of `nrep` back-to-back executions per timed call.

        Calls are dispatched asynchronously, each feeding its outputs back
        as the next call's donated output buffers, and blocked once at the
        end — slope over nrep isolates on-device time if dispatch pipelines.
        """
        import time as _t
        for _ in range(warmup):
            self.run_raw()
        ts = []
        self.dispatch_ts = []
        for _ in range(iters):
            outs = self._zeros()
            for zz in outs:
                zz.block_until_ready()
            t0 = _t.perf_counter()
            for _ in range(nrep):
                outs = self.sharded(*self._dev_in, *outs)
            self.dispatch_ts.append(_t.perf_counter() - t0)
            for o in outs:
                o.block_until_ready()
            ts.append(_t.perf_counter() - t0)
        return min(ts), ts


def kernel(**inputs):
    nsteps = T
    runner = _get_runner(nsteps)
    in_maps = _prep_inputs(**inputs, nsteps=nsteps)
    runner.set_inputs(in_maps)
    res = runner.results()
    # per core: out [nsteps*BL, V] rows (t, b) -> full (b, t, v)
    parts = []
    for c in range(NCORES):
        o = res[c]["out"].reshape(nsteps, BL, V)
        parts.append(o.transpose(1, 0, 2))
    full = np.concatenate(parts, axis=0)                  # (B, T, V)
    return np.ascontiguousarray(full)


# revision 28
# speedup vs baseline: 8.0966x; 1.0651x over previous
"""Trainium2 Bass kernel for the GRU session-decoder (nn_Decoder_12506944766179).

Strategy v3 (8 NeuronCores, SPMD, zero collectives):
  - Data-parallel over batch: core c owns batches 8c..8c+8 and runs the full
    GRU recurrence for them locally; no cross-core communication.
  - Hidden state (2H = 2048, host-permuted even/odd so max-pair pooling is
    h[0:1024] vs h[1024:2048]) lives partition-packed [128, 512] fp16:
    partition 32*j + b holds hidden quarter j of batch b.  The recurrent
    matmul gh = h @ w_hh.T runs as 4 concurrent PE column tiles
    (tile_position col tiling, 128x32 each); gate math runs at full
    128-partition width on DVE/ACT in fp16.
  - w_hh is fp16 and fully SBUF-resident.  n-gate matmuls are emitted first
    so ghn post-processing overlaps the r/z matmuls.
  - gi = emb[x] @ w_ih.T (+ biases) is precomputed for all steps into DRAM
    in a compact [(t, j, b), 1536] fp16 layout: stores and loads move as
    4 contiguous 24KiB descriptors per step (the old padded layout burned
    ~100 1KiB fragments per step).
  - Phase 2 computes max-pair -> lin2 -> +xe -> out_embed with w_out
    streamed chunk-by-chunk from DRAM (vocab-chunk outer loop), so only a
    2048-col fp16 chunk is SBUF-resident at a time.
  - Host side: all device buffers are placed with NamedSharding once;
    outputs are donated, so a steady-state call does no resharding.
"""

import os
import sys

sys.path.insert(0, "/opt/trn_rl_repo")

import numpy as np

import concourse.bass as bass
import concourse.mybir as mybir
import concourse.tile as tile
from concourse import bacc
from concourse.masks import make_identity

V, E, SH, H, B, T = 10004, 512, 1024, 1024, 64, 128
H2 = 2 * H                # 2048 hidden
G = 3 * H2                # 6144 gate columns
NCORES = 8
BL = B // NCORES          # 8 batches per core
WOFF = 32 - BL            # real batch b sits at partition 32*j + WOFF + b
NVC = (V + 511) // 512    # 20 vocab chunks of 512 (last one padded)
f32 = mybir.dt.float32
f16 = mybir.dt.float16
i32 = mybir.dt.int32
AF = mybir.ActivationFunctionType


def build(nsteps=T):
    nphases = int(os.environ.get("K_PHASES", "3"))
    tail = int(os.environ.get("K_TAIL", "1"))
    n1024 = int(os.environ.get("K_N1024", "0"))
    nrows = nsteps * BL   # token rows per core, (t, b) order

    nc = bacc.Bacc("TRN2", target_bir_lowering=False, debug=False,
                   num_devices=NCORES)

    emb = nc.declare_dram_parameter("emb", [V, E], f32, isOutput=False)
    idx = nc.declare_dram_parameter("idx", [nrows, 1], i32, isOutput=False)
    sesT_d = nc.declare_dram_parameter("sesT", [SH, BL], f32, isOutput=False)
    w1T_d = nc.declare_dram_parameter("w1T", [SH, H2], f32, isOutput=False)
    b1p_d = nc.declare_dram_parameter("b1p", [128, 512], f32, isOutput=False)
    wihT_d = nc.declare_dram_parameter("wihT", [E, G], f16, isOutput=False)
    bgi_d = nc.declare_dram_parameter("bgi", [128, G], f32, isOutput=False)
    whhT_d = nc.declare_dram_parameter("whhT", [H2, G], f16, isOutput=False)
    bhhn_d = nc.declare_dram_parameter("bhhn", [128, 512], f16, isOutput=False)
    w2T_d = nc.declare_dram_parameter("w2T", [H, E], f16, isOutput=False)
    b2t_d = nc.declare_dram_parameter("b2t", [128, E], f32, isOutput=False)
    wo2_d = nc.declare_dram_parameter("wo2", [NVC * 128, 2048], f16,
                                      isOutput=False)
    out = nc.declare_dram_parameter("out", [nrows, V], f32, isOutput=True)

    with tile.TileContext(nc) as tc:
        with (
            tc.tile_pool(name="wts", bufs=1) as wts,
            tc.tile_pool(name="dram", bufs=1, space="DRAM") as dram,
        ):
            # persistent small tiles
            ident16 = wts.tile([128, 128], f16, name="ident16")
            make_identity(nc, ident16[:])
            bhhn = wts.tile([128, 512], f16, name="bhhn")
            nc.sync.dma_start(bhhn[:], bhhn_d[:])
            # hT: stationary operand layout [128, (k:16, b:32)], fp16.
            # Pad columns (b >= BL) are zeroed once and never rewritten.
            hT = wts.tile([128, 16 * 32], f16, name="hT")
            nc.gpsimd.memset(hT[:], 0.0)
            # h state ping-pong (fp16, packed layout [128, 512])
            h_pp = [wts.tile([128, 512], f16, name=f"h{i}") for i in range(2)]

            gi_dram = dram.tile([nsteps * 32, 1536], f16, name="gi_dram")
            xe_dram = dram.tile([max(nrows, 128), E], f32, name="xe_dram")
            dec_dram = dram.tile([128, nsteps * 64], f16, name="dec_dram")

            def emit_transposes(h_tile, ps_pool, tag, bufs=2):
                """h_tile [128, 512] packed fp16 -> hT [128, (k, b<8)] fp16.

                4 full-width 128x128 PE transposes + one merged strided copy
                (chunk k = 4j + c4: hT col 32k+b <- ptr col 128*c4+32*j+b).
                """
                ptr = ps_pool.tile([128, 512], f16, name="ptr", tag=tag,
                                   bufs=bufs)
                for cp in range(4):
                    nc.tensor.transpose(
                        ptr[:, 128 * cp:128 * (cp + 1)],
                        h_tile[:, 128 * cp:128 * (cp + 1)],
                        ident16[:, :])
                nc.vector.tensor_copy(
                    hT[:].rearrange("p (j c4 w) -> p j c4 w",
                                    j=4, c4=4)[:, :, :, WOFF:32],
                    ptr[:].rearrange("p (c4 j w) -> p j c4 w",
                                     c4=4, j=4)[:, :, :, WOFF:32])

            # ---------------- phase 0: h0, xe gather, gi precompute ---------
            with (
                tc.tile_pool(name="p0sb", bufs=1) as p0,
                tc.tile_pool(name="p0ps", bufs=1, space="PSUM") as p0ps,
            ):
                ident = p0.tile([128, 128], f32, name="ident")
                make_identity(nc, ident[:])
                # h0 = tanh(ses @ w1p.T + b1p), packed layout
                ses_sb = p0.tile([128, 8 * BL], f32, name="ses_sb")
                nc.sync.dma_start(
                    ses_sb[:].rearrange("p (k b) -> p k b", k=8),
                    sesT_d.rearrange("(k p) b -> p k b", p=128))
                w1_sb = p0.tile([128, 8 * H2], f32, name="w1_sb")
                nc.sync.dma_start(
                    w1_sb[:].rearrange("p (k n) -> p k n", k=8),
                    w1T_d.rearrange("(k p) n -> p k n", p=128))
                b1p = p0.tile([128, 512], f32, name="b1p")
                nc.sync.dma_start(b1p[:], b1p_d[:])
                sesp = p0.tile([128, 8 * 32], f32, name="sesp")
                nc.gpsimd.memset(sesp[:], 0.0)
                nc.vector.tensor_copy(
                    sesp[:].rearrange("p (k w) -> p k w", k=8)[:, :, WOFF:32],
                    ses_sb[:].rearrange("p (k b) -> p k b", k=8))
                ps0 = p0ps.tile([128, 512], f32, name="ps0", bufs=1)
                for k in range(8):
                    for j in range(4):
                        nc.tensor.matmul(
                            ps0[32 * j:32 * (j + 1), :],
                            sesp[:, 32 * k:32 * (k + 1)],
                            w1_sb[:, k * H2 + 512 * j:
                                  k * H2 + 512 * (j + 1)],
                            start=(k == 0), stop=(k == 7),
                            tile_position=(0, 32 * j),
                            skip_group_check=True)
                nc.vector.tensor_add(ps0[:], ps0[:], b1p[:])
                nc.scalar.activation(h_pp[0][:], ps0[:], AF.Tanh)
                emit_transposes(h_pp[0], p0ps, "tr0", bufs=1)

                # gi for all rows
                wih = p0.tile([128, 4 * G], f16, name="wih")
                nc.sync.dma_start(
                    wih[:].rearrange("p (k n) -> p k n", k=4),
                    wihT_d.rearrange("(k p) n -> p k n", p=128))
                bgi = p0.tile([128, G], f32, name="bgi")
                nc.sync.dma_start(bgi[:], bgi_d[:])
                b2t = p0.tile([128, E], f32, name="b2t")
                nc.sync.dma_start(b2t[:], b2t_d[:])

                r0 = 0
                while nphases >= 1 and r0 < nrows:
                    R = min(128, nrows - r0)
                    ntl = R // BL
                    idxB = p0.tile([128, 1], i32, name="idxB", tag="idxB",
                                   bufs=2)
                    nc.sync.dma_start(idxB[0:R, :], idx[r0:r0 + R, :])
                    xeB = p0.tile([128, E], f32, name="xeB", tag="xeB", bufs=2)
                    nc.gpsimd.indirect_dma_start(
                        out=xeB[0:R, :], out_offset=None, in_=emb[:],
                        in_offset=bass.IndirectOffsetOnAxis(ap=idxB[0:R, :1],
                                                            axis=0))
                    # xe + b2 staged for the output residual
                    xeb2 = p0.tile([128, E], f32, name="xeb2", tag="xeb2",
                                   bufs=2)
                    nc.vector.tensor_add(xeb2[0:R, :], xeB[0:R, :],
                                         b2t[0:R, :])
                    nc.sync.dma_start(xe_dram[r0:r0 + R, :], xeb2[0:R, :])
                    # xe^T (fp16) for the gi matmul
                    ptx = p0ps.tile([128, 512], f32, name="ptx", tag="ptx",
                                    bufs=2)
                    for kc in range(4):
                        nc.tensor.transpose(
                            ptx[:, 128 * kc:128 * kc + R],
                            xeB[0:R, 128 * kc:128 * (kc + 1)],
                            ident[0:R, 0:R])
                    xeT = p0.tile([128, 512], f16, name="xeT", tag="xeT",
                                  bufs=2)
                    nc.vector.tensor_copy(
                        xeT[:].rearrange("p (kc r) -> p kc r", kc=4)[:, :, 0:R],
                        ptx[:].rearrange("p (kc r) -> p kc r", kc=4)[:, :, 0:R])
                    # gi_sb cols are (j, g, c) so per-(j, b) rows are
                    # 1536-contiguous for the compact store below
                    gi_sb = p0.tile([128, G], f16, name="gi_sb", tag="gi_sb",
                                    bufs=2)
                    for g in range(3):
                        psgi = p0ps.tile([128, 2048], f32, name="psgi",
                                         tag="psgi", bufs=1)
                        for kc in range(4):
                            for j in range(4):
                                nc.tensor.matmul(
                                    psgi[0:R, 512 * j:512 * (j + 1)],
                                    xeT[:, 128 * kc:128 * kc + R],
                                    wih[:, kc * G + g * H2 + 512 * j:
                                        kc * G + g * H2 + 512 * (j + 1)],
                                    start=(kc == 0), stop=(kc == 3))
                        nc.vector.tensor_add(
                            gi_sb[0:R, :].rearrange(
                                "r (j gg c) -> r j gg c", j=4, gg=3)[:, :, g],
                            psgi[0:R, :].rearrange("r (j c) -> r j c", j=4),
                            bgi[0:R, :].rearrange(
                                "r (j gg c) -> r j gg c", j=4, gg=3)[:, :, g])
                    # compact store: row t*32 + 4*b + j <- gi_sb row tl*BL+b,
                    # col slice j*1536 (8 contiguous 12KiB descriptors).
                    # Issued on the ACT DGE queue to decongest SP.
                    t0 = r0 // BL
                    for tl in range(ntl):
                        nc.scalar.dma_start(
                            gi_dram[(t0 + tl) * 32:(t0 + tl + 1) * 32, :]
                            .rearrange("(b j) c -> b j c", b=BL),
                            gi_sb[BL * tl:BL * (tl + 1), :].rearrange(
                                "b (j c) -> b j c", j=4))
                    r0 += R

            # ---------------- phase 1: recurrence ---------------------------
            if nphases >= 2:
              with (
                tc.tile_pool(name="msb", bufs=1) as msb,
                tc.tile_pool(name="mps", bufs=1, space="PSUM") as mps,
              ):
                whh = msb.tile([128, 16 * G], f16, name="whh")
                nc.scalar.dma_start(
                    whh[:].rearrange("p (k n) -> p k n", k=16),
                    whhT_d.rearrange("(k p) n -> p k n", p=128))

                h_cur = h_pp[0]
                for t in range(nsteps):
                    gi_t = msb.tile([128, 1536], f16, name="gi_t",
                                    tag="gi_t", bufs=2)
                    # zero the whole instance on the idle GPSIMD engine so
                    # pad partitions are initialized; the 4 contiguous-range
                    # loads below then overwrite the real partitions (real
                    # batch b lives at partition 32*j + WOFF + b)
                    nc.gpsimd.memset(gi_t[:], 0.0)
                    for j in range(4):
                        nc.sync.dma_start(
                            gi_t[32 * j + WOFF:32 * (j + 1), :],
                            gi_dram[32 * t:32 * (t + 1), :].rearrange(
                                "(b j) c -> b j c", b=BL)[:, j, :])
                    ghn = mps.tile([128, 512], f32, name="ghn", tag="ghn",
                                   bufs=2)
                    ghrz = mps.tile([128, 1024], f32, name="ghrz", tag="ghrz",
                                    bufs=2)

                    def gate_mms(dst, c0, q, rhs0, nn=512):
                        # open the accumulation group with a full-width
                        # identity matmul that injects the bias/gi term into
                        # PSUM (removes the DVE add from the critical path),
                        # then accumulate the 16 col-tiled h @ w_hh chunks
                        nc.tensor.matmul(dst[:, c0:c0 + nn], ident16[:, :],
                                         rhs0, start=True, stop=False,
                                         skip_group_check=True)
                        for k in range(16):
                            for j in range(4):
                                nc.tensor.matmul(
                                    dst[32 * j:32 * (j + 1), c0:c0 + nn],
                                    hT[:, 32 * k:32 * (k + 1)],
                                    whh[:, k * G + (j * 3 + q) * 512:
                                        k * G + (j * 3 + q) * 512 + nn],
                                    start=False, stop=(k == 15),
                                    tile_position=(0, 32 * j),
                                    skip_group_check=True)

                    # group order (n, r, z): the n/r-dependent chain
                    # (sigmoid -> n pre-act -> tanh) overlaps the z matmuls,
                    # so only z-sigmoid + 2 DVE ops trail the last matmul
                    gate_mms(ghn, 0, 2, bhhn[:])
                    if n1024:
                        # merged r+z: N=1024 fp16 moving operand (r and z
                        # weight columns are adjacent per quarter j)
                        gate_mms(ghrz, 0, 0, gi_t[:, 0:1024], nn=1024)
                    else:
                        gate_mms(ghrz, 0, 0, gi_t[:, 0:512])
                    if not tail:
                        continue
                    r = msb.tile([128, 512], f16, name="r", tag="r")
                    nc.scalar.activation(r[:], ghrz[:, 0:512], AF.Sigmoid)
                    # n = tanh(r * (ghn + b_hh_n) + gi_n), in place
                    ng = msb.tile([128, 512], f16, name="ng", tag="ng")
                    nc.vector.tensor_mul(ng[:], r[:], ghn[:])
                    nc.vector.tensor_add(ng[:], ng[:], gi_t[:, 1024:1536])
                    nc.scalar.activation(ng[:], ng[:], AF.Tanh)
                    # dd = h - n can also run during the z matmuls
                    dd = msb.tile([128, 512], f16, name="dd", tag="dd")
                    nc.vector.tensor_sub(dd[:], h_cur[:], ng[:])
                    if not n1024:
                        gate_mms(ghrz, 512, 1, gi_t[:, 512:1024])
                    z = msb.tile([128, 512], f16, name="z", tag="z")
                    nc.scalar.activation(z[:], ghrz[:, 512:1024], AF.Sigmoid)
                    # h_new = n + z * (h - n)
                    nc.vector.tensor_mul(dd[:], z[:], dd[:])
                    h_new = h_pp[(t + 1) % 2]
                    nc.vector.tensor_add(h_new[:], ng[:], dd[:])
                    emit_transposes(h_new, mps, "tr")
                    dct = msb.tile([128, 256], f16, name="dct", tag="dct",
                                   bufs=1)
                    nc.vector.tensor_max(dct[:], hT[:, 0:256], hT[:, 256:512])
                    nc.sync.dma_start(
                        dec_dram[:, 64 * t:64 * (t + 1)].rearrange(
                            "p (k b) -> p k b", k=8),
                        dct[:].rearrange("p (k w) -> p k w",
                                         k=8)[:, :, WOFF:32])
                    h_cur = h_new

            # ---------------- phase 2: output projections --------------------
            if nphases >= 3:
              with (
                tc.tile_pool(name="p2sb", bufs=1) as p2,
                tc.tile_pool(name="p2ps", bufs=1, space="PSUM") as p2ps,
              ):
                w2 = p2.tile([128, 8 * E], f16, name="w2")
                nc.sync.dma_start(
                    w2[:].rearrange("p (k n) -> p k n", k=8),
                    w2T_d.rearrange("(k p) n -> p k n", p=128))

                # stage A: per block, lin2 + xe residual, transposed fp16
                nblk = (nrows + 127) // 128
                d2T = [p2.tile([128, 512], f16, name=f"d2T{i}")
                       for i in range(nblk)]
                for blk in range(nblk):
                    r0 = blk * 128
                    R = min(128, nrows - r0)
                    ntl = R // BL
                    t0 = r0 // BL
                    dec_sb = p2.tile([128, 16 * 64], f16, name="dec_sb",
                                     tag="dec_sb", bufs=2)
                    nc.sync.dma_start(dec_sb[:, 0:ntl * 64],
                                      dec_dram[:, t0 * 64:(t0 + ntl) * 64])
                    # repack (tl, k, b) -> (k, tl, b) so lhsT slices are
                    # contiguous
                    dec2_sb = p2.tile([128, 16 * 64], f16, name="dec2_sb",
                                      tag="dec2_sb", bufs=2)
                    nc.vector.tensor_copy(
                        dec2_sb[:, 0:ntl * 64].rearrange(
                            "p (k tl b) -> p k tl b", k=8, tl=ntl),
                        dec_sb[:, 0:ntl * 64].rearrange(
                            "p (tl k b) -> p k tl b", tl=ntl, k=8))
                    ps2 = p2ps.tile([128, 512], f32, name="ps2", tag="ps2",
                                    bufs=2)
                    for k in range(8):
                        nc.tensor.matmul(
                            ps2[0:R, :],
                            dec2_sb[:, k * ntl * BL:(k + 1) * ntl * BL],
                            w2[:, 512 * k:512 * (k + 1)],
                            start=(k == 0), stop=(k == 7))
                    xe_sb = p2.tile([128, E], f32, name="xe_sb", tag="xe_sb",
                                    bufs=2)
                    nc.sync.dma_start(xe_sb[0:R, :], xe_dram[r0:r0 + R, :])
                    dec2 = p2.tile([128, E], f16, name="dec2", tag="dec2",
                                   bufs=2)
                    nc.vector.tensor_add(dec2[0:R, :], ps2[0:R, :],
                                         xe_sb[0:R, :])
                    pst = p2ps.tile([128, 512], f16, name="pst", tag="pst",
                                    bufs=2)
                    for kc in range(4):
                        nc.tensor.transpose(
                            pst[:, 128 * kc:128 * kc + R],
                            dec2[0:R, 128 * kc:128 * (kc + 1)],
                            ident16[0:R, 0:R])
                    nc.vector.tensor_copy(
                        d2T[blk][:].rearrange("p (kc r) -> p kc r",
                                              kc=4)[:, :, 0:R],
                        pst[:].rearrange("p (kc r) -> p kc r",
                                         kc=4)[:, :, 0:R])

                # stage B: vocab chunks processed in pairs (1024 cols per
                # store), w_out streamed on the ACT queue, stores on SP
                for vp in range(NVC // 2):
                    n0 = 1024 * vp
                    NN = min(1024, V - n0)
                    wch = p2.tile([128, 4096], f16, name="wch", tag="wch",
                                  bufs=2)
                    nc.scalar.dma_start(
                        wch[:].rearrange("p (two c) -> p two c", two=2),
                        wo2_d[256 * vp:256 * (vp + 1), :].rearrange(
                            "(two p) c -> p two c", p=128))
                    for blk in range(nblk):
                        r0 = blk * 128
                        R = min(128, nrows - r0)
                        lgs = p2.tile([128, 1024], f32, name="lgs", tag="lgs",
                                      bufs=3)
                        for half in range(2):
                            psl = p2ps.tile([128, 512], f32, name="psl",
                                            tag="psl", bufs=2)
                            for kc in range(4):
                                nc.tensor.matmul(
                                    psl[0:R, 0:512],
                                    d2T[blk][:, 128 * kc:128 * kc + R],
                                    wch[:, 2048 * half + 512 * kc:
                                        2048 * half + 512 * (kc + 1)],
                                    start=(kc == 0), stop=(kc == 3))
                            h0c = 512 * half
                            hw = min(512, max(0, NN - h0c))
                            if hw > 0:
                                nc.vector.tensor_copy(
                                    lgs[0:R, h0c:h0c + hw],
                                    psl[0:R, 0:hw])
                        nc.sync.dma_start(out[r0:r0 + R, n0:n0 + NN],
                                          lgs[0:R, 0:NN])

    nc.compile()
    return nc


# ---------------------------------------------------------------------------
# host side
# ---------------------------------------------------------------------------

def _prep_inputs(ses_encoding, x, x_lens, emb_table, w1, b1, w_ih, w_hh,
                 b_ih, b_hh, w2, b2, w_out, nsteps=T):
    f = np.float32
    h = np.float16
    ses = np.asarray(ses_encoding, f)
    emb = np.ascontiguousarray(np.asarray(emb_table, f))
    w1 = np.asarray(w1, f)
    b1 = np.asarray(b1, f)
    w_ih = np.asarray(w_ih, f)
    w_hh = np.asarray(w_hh, f)
    b_ih = np.asarray(b_ih, f)
    b_hh = np.asarray(b_hh, f)
    w2 = np.asarray(w2, f)
    b2 = np.asarray(b2, f)
    w_out = np.asarray(w_out, f)
    x = np.asarray(x).astype(np.int32)

    hperm = np.concatenate([np.arange(0, H2, 2), np.arange(1, H2, 2)])

    # shared weights (identical on every core)
    w1T = np.ascontiguousarray(w1[hperm % H, :].T)                 # (SH, 2048)
    b1p = np.ascontiguousarray(
        np.repeat(b1[hperm % H].reshape(4, 512), 32, axis=0)).astype(f)
    gcols = np.concatenate([g * H2 + hperm for g in range(3)])     # (6144,)
    wihT = np.ascontiguousarray(w_ih[gcols, :].T.astype(h))        # (512, 6144)
    bias_v = (b_ih[gcols] +
              np.where(np.arange(G) < 2 * H2, b_hh[gcols], 0.0)).astype(f)
    # reorder (g, j, c) -> (j, g, c) for the compact gi layout
    bias_jgc = np.ascontiguousarray(
        bias_v.reshape(3, 4, 512).transpose(1, 0, 2).reshape(G))
    bgi = np.ascontiguousarray(np.tile(bias_jgc, (128, 1)))        # (128, 6144)
    grows = np.concatenate([q * H2 + hperm[512 * j:512 * (j + 1)]
                            for j in range(4) for q in range(3)])  # (6144,)
    whhT = np.ascontiguousarray(w_hh[grows][:, hperm].T.astype(h)) # (2048,6144)
    bhhn = np.ascontiguousarray(
        np.repeat(b_hh[2 * H2 + hperm].reshape(4, 512), 32, axis=0)).astype(h)
    w2T = np.ascontiguousarray(w2.T.astype(h))                     # (1024, 512)
    b2t = np.ascontiguousarray(np.tile(b2.reshape(1, E), (128, 1))).astype(f)
    # w_out in vocab-chunk layout: wo2[vc*128 + p, kc*512 + c]
    #   = w_out[512*vc + c, 128*kc + p]
    wo_pad = np.zeros((NVC * 512, E), f)
    wo_pad[:V] = w_out
    wo2 = np.ascontiguousarray(
        wo_pad.reshape(NVC, 512, 4, 128).transpose(0, 3, 2, 1)
        .reshape(NVC * 128, 2048).astype(h))

    in_maps = []
    for c in range(NCORES):
        xloc = x[BL * c:BL * (c + 1), :nsteps]                     # (8, t)
        idxs = np.ascontiguousarray(xloc.T.reshape(nsteps * BL, 1))
        sesT = np.ascontiguousarray(ses[0, BL * c:BL * (c + 1), :].T)
        in_maps.append(dict(
            emb=emb, idx=idxs, sesT=sesT, w1T=w1T, b1p=b1p,
            wihT=wihT, bgi=bgi, whhT=whhT, bhhn=bhhn, w2T=w2T, b2t=b2t,
            wo2=wo2))
    return in_maps


_CACHED = {}


def _get_runner(nsteps=T, nreps=1):
    key = (nsteps, nreps)
    if key not in _CACHED:
        nc = _CACHED.get(("nc", nsteps))
        if nc is None:
            nc = build(nsteps)
            _CACHED[("nc", nsteps)] = nc
        _CACHED[key] = _SpmdRunner(nc, NCORES, nreps=nreps)
    return _CACHED[key]


class _SpmdRunner:
    def __init__(self, nc, n_cores, nreps=1):
        import jax
        import jax.numpy as jnp
        from jax.sharding import Mesh, PartitionSpec, NamedSharding
        from jax.experimental.shard_map import shard_map
        from concourse.bass2jax import (_bass_exec_p, partition_id_tensor,
                                        install_neuronx_cc_hook)
        self.jax = jax
        self.jnp = jnp
        install_neuronx_cc_hook()
        self.nc = nc
        self.n_cores = n_cores
        self.nreps = nreps
        in_names, out_names, out_avals = [], [], []
        pname = nc.partition_id_tensor.name if nc.partition_id_tensor else None
        for alloc in nc.m.functions[0].allocations:
            if not isinstance(alloc, mybir.MemoryLocationSet):
                continue
            name = alloc.memorylocations[0].name
            if alloc.kind == "ExternalInput":
                if name != pname:
                    in_names.append(name)
            elif alloc.kind == "ExternalOutput":
                out_names.append(name)
                out_avals.append(jax.core.ShapedArray(
                    tuple(alloc.tensor_shape), mybir.dt.np(alloc.dtype)))
        self.in_names, self.out_names, self.out_avals = \
            in_names, out_names, out_avals
        n_params, n_outs = len(in_names), len(out_avals)
        all_in = in_names + out_names + ([pname] if pname else [])

        def _body(*args):
            # neuronx_cc_hook requires bass_exec operands to be the jit
            # parameters in exact order, so exactly one exec per jitted call.
            operands = list(args)
            if pname is not None:
                operands.append(partition_id_tensor())
            return tuple(_bass_exec_p.bind(
                *operands, out_avals=tuple(out_avals),
                in_names=tuple(all_in), out_names=tuple(out_names),
                lowering_input_output_aliases=(),
                sim_require_finite=False, sim_require_nnan=False, nc=nc))

        devices = jax.devices()[:n_cores]
        mesh = Mesh(np.asarray(devices), ("core",))
        # Shard inputs/outputs along axis 0 across the cores ONCE, outside
        # the dispatch path — otherwise every jitted call reshards ~500MB of
        # weights plus the output buffer from device 0 to all 8 cores.
        self.sharding = NamedSharding(mesh, PartitionSpec("core"))
        self.donate = tuple(range(n_params, n_params + n_outs))
        self.sharded = jax.jit(
            shard_map(_body, mesh=mesh,
                      in_specs=(PartitionSpec("core"),) * (n_params + n_outs),
                      out_specs=(PartitionSpec("core"),) * n_outs,
                      check_rep=False),
            donate_argnums=self.donate, keep_unused=True)

    def set_inputs(self, in_maps):
        jax = self.jax
        per_core = [[np.ascontiguousarray(m[n]) for n in self.in_names]
                    for m in in_maps]
        concat = [np.concatenate([per_core[c][i] for c in range(self.n_cores)],
                                 axis=0) for i in range(len(self.in_names))]
        self._dev_in = [jax.device_put(a, self.sharding) for a in concat]
        for a in self._dev_in:
            a.block_until_ready()

    def share_inputs(self, other):
        self._dev_in = other._dev_in

    def _zeros(self):
        return [self.jax.device_put(
            np.zeros((self.n_cores * av.shape[0], *av.shape[1:]), av.dtype),
            self.sharding) for av in self.out_avals]

    def run_raw(self):
        outs = self.sharded(*self._dev_in, *self._zeros())
        for o in outs:
            o.block_until_ready()
        return outs

    def results(self):
        outs = self.run_raw()
        res = []
        for c in range(self.n_cores):
            res.append({n: np.asarray(outs[i]).reshape(
                self.n_cores, *self.out_avals[i].shape)[c]
                for i, n in enumerate(self.out_names)})
        return res

    def time(self, iters=10, warmup=2):
        return self.time_chain(1, iters=iters, warmup=warmup)

    def time_chain(self, nrep, iters=10, warmup=2):
        """Wall time of `nrep` back-to-back executions per timed call.

        Calls are dispatched asynchronously, each feeding its outputs back
        as the next call's donated output buffers, and blocked once at the
        end — slope over nrep isolates on-device time if dispatch pipelines.
        """
        import time as _t
        for _ in range(warmup):
            self.run_raw()
        ts = []
        self.dispatch_ts = []
        for _ in range(iters):
            outs = self._zeros()
            for zz in outs:
                zz.block_until_ready()
            t0 = _t.perf_counter()
            for _ in range(nrep):
                outs = self.sharded(*self._dev_in, *outs)
            self.dispatch_ts.append(_t.perf_counter() - t0)
            for o in outs:
                o.block_until_ready()
            ts.append(_t.perf_counter() - t0)
        return min(ts), ts


def kernel(**inputs):
    nsteps = T
    runner = _get_runner(nsteps)
    in_maps = _prep_inputs(**inputs, nsteps=nsteps)
    runner.set_inputs(in_maps)
    res = runner.results()
    # per core: out [nsteps*BL, V] rows (t, b) -> full (b, t, v)
    parts = []
    for c in range(NCORES):
        o = res[c]["out"].reshape(nsteps, BL, V)
        parts.append(o.transpose(1, 0, 2))
    full = np.concatenate(parts, axis=0)                  # (B, T, V)
    return np.ascontiguousarray(full)
